# revision 25
# baseline (speedup 1.0000x reference)
"""Trainium2 Bass kernel for a local-attention transformer block (v3, fp16).

Computes, per batch element (one NeuronCore each, 8 cores):
  ss = silu(t_emb) @ time_w + time_b ;  scale, shift = split(ss)
  y  = LN(x) * (1+scale) + shift                       (ln1 g/b host-folded)
  q,k,v = y @ qkv_w + qkv_b  (heads=8, d=64)
  attn: each 128-token window attends to [prev|cur|next] windows
  x1 = x + attn @ proj_w + proj_b
  out = x1 + gelu(LN2(x1) @ w1 + b1') @ w2 + b2        (ln2 g/b folded into w1/b1)

v3 strategy (evolved from the 2.18 ms fp16 baseline):
  - All GEMMs fp16 (measured: fp16=bf16=fp8 all stream 216 ns per N=512 matmul;
    DoubleRow's 256-col LDWEIGHTS doesn't background-load, so fp8 gains nothing).
    Weights stored [128, n_chunks, out] fp16, activations transposed to
    [128, n_chunks, 512] fp16 chunk tiles.
  - Attention key-block-major: per (head, key block j) ONE sim matmul of
    N<=384 (q windows j-1..j+1, keys on partitions), exp into an E tile
    reused by 3 AV windows; AV accumulates [65, 4win, 128] PSUM per head
    (ones column folded into v_aug produces softmax denominators).
  - LN transposes x_hat fp16 via ONE batched DMA transpose per token tile
    ([128,512] -> [128,4,128], same 1.2us as a 128x128 transpose); modulate
    fused into a per-chunk tensor_scalar on the transposed side.
  - ACT engine runs ONLY Exp/Gelu/Sqrt (no Identity copies, no DMA issue):
    PSUM evacuations + bias adds on DVE (cross-partition DVE copies verified),
    per-head softmax reciprocal on DVE directly from PSUM row 64.
  - All DMA on the sync ring (transposes batched); scalar ring unused so the
    ACT queue stays clean.
"""

import numpy as np
from contextlib import ExitStack

import concourse.bass as bass
import concourse.tile as tile
from concourse import bacc, mybir
from concourse import bass_utils

F32 = mybir.dt.float32
F16 = mybir.dt.float16
BF16 = mybir.dt.bfloat16
AF = mybir.ActivationFunctionType
AL = mybir.AluOpType

DIM = 512
HEADS = 8
HD = 64
FF = 2048
WIN = 128
B = 8
NTOK = 8192
EPS = 1e-5
GRP = 512  # tokens per group (4 windows)
SIMSCALE = float(HD) ** -0.5


def _col_view(dram_ap, offset, ncol):
    """AP reading dram vector [128*ncol] as [128, ncol] feature-major columns."""
    return bass.AP(tensor=dram_ap.tensor, offset=offset, ap=[[1, 128], [128, ncol]])


def _bcast_row(dram_ap, offset, n):
    """AP reading dram vector [n] broadcast across 128 partitions."""
    return bass.AP(tensor=dram_ap.tensor, offset=offset, ap=[[0, 128], [1, n]])


def build(n_tok=NTOK):
    n_groups = n_tok // GRP
    nW = n_tok // WIN
    nc = bacc.Bacc("TRN2", target_bir_lowering=False, debug=False)

    x_d = nc.dram_tensor("x", [n_tok, DIM], F32, kind="ExternalInput")
    arow_d = nc.dram_tensor("arow", [DIM], F32, kind="ExternalInput")
    crow_d = nc.dram_tensor("crow", [DIM], F32, kind="ExternalInput")
    qkvw_d = nc.dram_tensor("qkvw", [128, 4, 3 * DIM], F16, kind="ExternalInput")
    qkb_d = nc.dram_tensor("qkb", [2 * DIM], F32, kind="ExternalInput")
    vb_d = nc.dram_tensor("vb", [DIM], F32, kind="ExternalInput")
    projw_d = nc.dram_tensor("projw", [128, 4, DIM], F16, kind="ExternalInput")
    projb_d = nc.dram_tensor("projb", [DIM], F32, kind="ExternalInput")
    w1_d = nc.dram_tensor("w1", [128, 4, FF], F16, kind="ExternalInput")
    b1_d = nc.dram_tensor("b1", [FF], F32, kind="ExternalInput")
    w2_d = nc.dram_tensor("w2", [128, 16, DIM], F16, kind="ExternalInput")
    b2_d = nc.dram_tensor("b2", [DIM], F32, kind="ExternalInput")
    out_d = nc.dram_tensor("out", [n_tok, DIM], F32, kind="ExternalOutput")

    with tile.TileContext(nc) as tc:
        with ExitStack() as ctx:
            consts = ctx.enter_context(tc.tile_pool(name="consts", bufs=1))
            xp = ctx.enter_context(tc.tile_pool(name="xp", bufs=2))
            xpbp = ctx.enter_context(tc.tile_pool(name="xpbp", bufs=1))
            xhp = ctx.enter_context(tc.tile_pool(name="xhp", bufs=1))
            ytp = ctx.enter_context(tc.tile_pool(name="ytp", bufs=2))
            qp = ctx.enter_context(tc.tile_pool(name="qp", bufs=2))
            kp = ctx.enter_context(tc.tile_pool(name="kp", bufs=2))
            vp = ctx.enter_context(tc.tile_pool(name="vp", bufs=2))
            ep = ctx.enter_context(tc.tile_pool(name="ep", bufs=1))
            astp = ctx.enter_context(tc.tile_pool(name="astp", bufs=1))
            a16p = ctx.enter_context(tc.tile_pool(name="a16p", bufs=1))
            x1p = ctx.enter_context(tc.tile_pool(name="x1p", bufs=1))
            h2tp = ctx.enter_context(tc.tile_pool(name="h2tp", bufs=1))
            gelp = ctx.enter_context(tc.tile_pool(name="gelp", bufs=1))
            op = ctx.enter_context(tc.tile_pool(name="op", bufs=2))
            sp = ctx.enter_context(tc.tile_pool(name="sp", bufs=2))
            rp = ctx.enter_context(tc.tile_pool(name="rp", bufs=1))
            tp = ctx.enter_context(tc.tile_pool(name="tp", bufs=2))
            dp = ctx.enter_context(tc.tile_pool(name="dp", bufs=2, space="DRAM"))
            ps_g = ctx.enter_context(tc.tile_pool(name="ps_g", bufs=3, space="PSUM"))
            ps_s = ctx.enter_context(tc.tile_pool(name="ps_s", bufs=3, space="PSUM"))
            ps_a = ctx.enter_context(tc.tile_pool(name="ps_a", bufs=2, space="PSUM"))

            # ---- constants ----
            qkvw_sb = consts.tile([128, 4, 3 * DIM], F16, name="qkvw_sb")
            nc.sync.dma_start(qkvw_sb[:], qkvw_d[:, :, :])
            projw_sb = consts.tile([128, 4, DIM], F16, name="projw_sb")
            nc.sync.dma_start(projw_sb[:], projw_d[:, :, :])
            w1_sb = consts.tile([128, 4, FF], F16, name="w1_sb")
            nc.sync.dma_start(w1_sb[:], w1_d[:, :, :])
            w2_sb = consts.tile([128, 16, DIM], F16, name="w2_sb")
            nc.sync.dma_start(w2_sb[:], w2_d[:, :, :])

            arow_col = consts.tile([128, 4], F32, name="arow_col")
            nc.sync.dma_start(arow_col[:], _col_view(arow_d.ap(), 0, 4))
            crow_col = consts.tile([128, 4], F32, name="crow_col")
            nc.sync.dma_start(crow_col[:], _col_view(crow_d.ap(), 0, 4))
            qkb_sb = consts.tile([128, 8], F32, name="qkb_sb")
            nc.sync.dma_start(qkb_sb[:], _col_view(qkb_d.ap(), 0, 8))
            b1_sb = consts.tile([128, 16], F32, name="b1_sb")
            nc.sync.dma_start(b1_sb[:], _col_view(b1_d.ap(), 0, 16))
            vb_bc = consts.tile([128, DIM], F32, name="vb_bc")
            nc.sync.dma_start(vb_bc[:], _bcast_row(vb_d.ap(), 0, DIM))
            projb_bc = consts.tile([128, DIM], F32, name="projb_bc")
            nc.sync.dma_start(projb_bc[:], _bcast_row(projb_d.ap(), 0, DIM))
            b2_bc = consts.tile([128, DIM], F32, name="b2_bc")
            nc.sync.dma_start(b2_bc[:], _bcast_row(b2_d.ap(), 0, DIM))
            eps_t = consts.tile([128, 1], F32, name="eps_t")
            nc.vector.memset(eps_t[:], EPS)

            stages = {}   # g -> dict of tiles
            e_tiles = {}  # (h, j) -> E tile

            def qkv_ln(g):
                """x load + LN1 + transpose + modulate -> y16 (no matmuls).

                Emitted one group early so its ACT sqrt isn't queued behind
                the attention exps and PE always has QKV work ready."""
                st = {}
                xts, xpbs = [], []
                mv = tp.tile([128, 4, 2], F32, name=f"mv1_{g}", tag="mv1")
                for t in range(4):
                    xt = xp.tile([128, DIM], F32, name=f"x_{g}_{t}", tag=f"x{t % 2}")
                    nc.sync.dma_start(xt[:], x_d[(g * 4 + t) * 128:(g * 4 + t + 1) * 128, :])
                    stats = tp.tile([128, 6], F32, name=f"st_{g}_{t}", tag=f"st{t}")
                    nc.vector.bn_stats(stats[:], xt[:])
                    nc.vector.bn_aggr(mv[:, t:t + 1, :], stats[:])
                    xts.append(xt)
                std = tp.tile([128, 4], F32, name=f"sd_{g}", tag="sd1")
                nc.scalar.activation(std[:], mv[:, :, 1:2], AF.Sqrt, bias=eps_t[:])
                rs = tp.tile([128, 4], F32, name=f"rs_{g}", tag="rs1")
                nc.vector.reciprocal(rs[:], std[:])
                xhT = ytp.tile([128, 4, GRP], F16, name=f"xhT_{g}", tag="xhT")
                for t in range(4):
                    xh = xhp.tile([128, DIM], F16, name=f"xh_{g}_{t}", tag=f"xh{t}")
                    nc.vector.tensor_scalar(xh[:], xts[t][:], mv[:, t:t + 1, 0:1], rs[:, t:t + 1],
                                            op0=AL.subtract, op1=AL.mult)
                    # x + projb precomputed (on idle GpSimd) so x tiles die early
                    xpb = xpbp.tile([128, DIM], F32, name=f"xpb_{g}_{t}", tag=f"xpb{t}")
                    nc.gpsimd.tensor_tensor(xpb[:], xts[t][:], projb_bc[:], op=AL.add)
                    xpbs.append(xpb)
                    # one batched transpose per token tile (sync ring)
                    nc.sync.dma_start_transpose(xhT[:, :, t * 128:(t + 1) * 128], xh[:])
                st["xpb"] = xpbs
                # modulate per chunk (arow/crow are per-partition on transposed side)
                y16 = ytp.tile([128, 4, GRP], F16, name=f"y16_{g}", tag="y16")
                for c in range(4):
                    nc.vector.tensor_scalar(y16[:, c, :], xhT[:, c, :],
                                            arow_col[:, c:c + 1], crow_col[:, c:c + 1],
                                            op0=AL.mult, op1=AL.add)
                st["y16"] = y16
                return st

            def qkv_mm(g):
                st = stages[g]
                y16 = st["y16"]
                # Q,K: weight-stationary, out feature-major [128 f, 512 tok]
                q_t, k_t = [], []
                for m in range(8):
                    P = ps_g.tile([128, GRP], F32, name=f"Pqk_{g}_{m}", tag="gemm")
                    for c in range(4):
                        nc.tensor.matmul(P[:], qkvw_sb[:, c, m * 128:(m + 1) * 128],
                                         y16[:, c, :], start=(c == 0), stop=(c == 3))
                    pool = qp if m < 4 else kp
                    nm = f"q_{g}_{m}" if m < 4 else f"k_{g}_{m-4}"
                    tg = f"q{m}" if m < 4 else f"k{m-4}"
                    sb = pool.tile([128, GRP], F16, name=nm, tag=tg)
                    nc.vector.tensor_scalar_add(sb[:], P[:], qkb_sb[:, m:m + 1])
                    (q_t if m < 4 else k_t).append(sb)
                st["q"], st["k"] = q_t, k_t
                # V: activation-stationary, out token-major + ones col
                v_t = []
                for t in range(4):
                    P = ps_g.tile([128, DIM], F32, name=f"Pv_{g}_{t}", tag="gemm")
                    for c in range(4):
                        nc.tensor.matmul(P[:], y16[:, c, t * 128:(t + 1) * 128],
                                         qkvw_sb[:, c, 2 * DIM:3 * DIM],
                                         start=(c == 0), stop=(c == 3))
                    vt = vp.tile([128, HEADS, HD + 1], F16, name=f"v_{g}_{t}", tag=f"v{t}")
                    nc.vector.memset(vt[:, :, HD:HD + 1], 1.0)
                    nc.vector.tensor_tensor(
                        vt[:, :, 0:HD],
                        P[:].rearrange("p (h d) -> p h d", h=HEADS),
                        vb_bc[:].rearrange("p (h d) -> p h d", h=HEADS),
                        op=AL.add)
                    v_t.append(vt)
                st["v"] = v_t

            def attn_stage(gp):
                # key blocks computed this phase
                j_list = [j for j in range(4 * gp + 1, 4 * gp + 5) if j < nW]
                if gp == 0:
                    j_list = [0] + j_list
                attn_f16 = {}
                sums_dr = dp.tile([8, GRP], F32, name=f"sums_{gp}", tag="sums")
                for hp in range(4):
                    heads = (2 * hp, 2 * hp + 1)
                    af = astp.tile([128, GRP], F16, name=f"af_{gp}_{hp}", tag=f"af{hp}")
                    attn_f16[hp] = af
                    P_av = {}
                    for h in heads:
                        P_av[h] = ps_a.tile([65, 4, 128], F32, name=f"Pav_{gp}_{h}", tag="av")
                    for j in j_list:
                        gj, sj = divmod(j, 4)
                        qlo = max(0, j - 1)
                        qhi = min(nW - 1, j + 1)
                        ncols = (qhi - qlo + 1) * WIN
                        for h in heads:
                            off = (h % 2) * 64
                            P_sim = ps_s.tile([128, 384], F32, name=f"Ps_{gp}_{h}_{j}", tag="sim")
                            # q columns may span two group tiles -> split segments
                            w0 = qlo
                            while w0 <= qhi:
                                gq = w0 // 4
                                wend = min(qhi, gq * 4 + 3)
                                c0 = (w0 % 4) * WIN
                                c1 = (wend % 4 + 1) * WIN
                                dst0 = (w0 - qlo) * WIN
                                nc.tensor.matmul(
                                    P_sim[:, dst0:dst0 + (c1 - c0)],
                                    stages[gj]["k"][hp][off:off + 64, sj * 128:(sj + 1) * 128],
                                    stages[gq]["q"][hp][off:off + 64, c0:c1],
                                    start=True, stop=True)
                                w0 = wend + 1
                            E = ep.tile([128, 384], F16, name=f"E_{gp}_{h}_{j}",
                                        tag=f"E{h}_{j % 3}")
                            nc.scalar.activation(E[:, 0:ncols], P_sim[:, 0:ncols],
                                                 AF.Exp, scale=SIMSCALE)
                            e_tiles[(h, j)] = E
                        # AV for completed windows
                        av_ws = []
                        if 4 * gp <= j - 1 <= 4 * gp + 3:
                            av_ws.append(j - 1)
                        if j == j_list[-1] and j == nW - 1:
                            av_ws.append(nW - 1)
                        for w in av_ws:
                            jjs = [jj for jj in (w - 1, w, w + 1) if 0 <= jj < nW]
                            for h in heads:
                                for ji, jj in enumerate(jjs):
                                    gjj, sjj = divmod(jj, 4)
                                    colofs = (w - max(0, jj - 1)) * WIN
                                    nc.tensor.matmul(
                                        P_av[h][:, w % 4, :],
                                        stages[gjj]["v"][sjj][:, h, :],
                                        e_tiles[(h, jj)][:, colofs:colofs + WIN],
                                        start=(ji == 0), stop=(ji == len(jjs) - 1))
                    # evacuate PSUM: unnormalized attn (DVE when aligned) + sums
                    for h in heads:
                        off = (h % 2) * 64
                        src = P_av[h][0:64, :, :].rearrange("p a b -> p (a b)")
                        if h % 2 == 0:
                            nc.vector.tensor_copy(af[0:64, :], src)
                        else:
                            nc.scalar.activation(af[64:128, :], src, AF.Identity)
                        ss = sp.tile([1, GRP], F32, name=f"ss_{gp}_{h}", tag="ss")
                        nc.scalar.activation(ss[:],
                                             P_av[h][64:65, :, :].rearrange("p a b -> p (a b)"),
                                             AF.Identity)
                        nc.sync.dma_start(sums_dr[h:h + 1, :], ss[:])
                # one batched reciprocal + broadcast (DRAM round trip), normalize -> fp16
                sums_sb = sp.tile([8, GRP], F32, name=f"sums_sb_{gp}", tag="sums_sb")
                nc.sync.dma_start(sums_sb[:], sums_dr[:])
                r_sb = sp.tile([8, GRP], F32, name=f"r_sb_{gp}", tag="r_sb")
                nc.vector.reciprocal(r_sb[:], sums_sb[:])
                r_dr = dp.tile([8, GRP], F32, name=f"r_dr_{gp}", tag="rd")
                nc.sync.dma_start(r_dr[:], r_sb[:])
                a16 = a16p.tile([128, 4, GRP], F16, name=f"a16_{gp}", tag="a16")
                for hp in range(4):
                    rbc = rp.tile([128, GRP], F32, name=f"rbc_{gp}_{hp}", tag=f"rbc{hp}")
                    src = bass.AP(tensor=r_dr[:].tensor, offset=r_dr[:].offset + 2 * hp * GRP,
                                  ap=[[GRP, 2], [0, 64], [1, GRP]])
                    nc.sync.dma_start(rbc[:], src)
                    nc.vector.tensor_tensor(a16[:, hp, :], attn_f16[hp][:], rbc[:], op=AL.mult)
                return a16

            def proj_mlp_stage(gp, a16):
                cur = stages[gp]
                # proj + residual -> x1 (token-major f32)
                x1_t = []
                for t in range(4):
                    P = ps_g.tile([128, DIM], F32, name=f"Ppr_{gp}_{t}", tag="gemm")
                    for c in range(4):
                        nc.tensor.matmul(P[:], a16[:, c, t * 128:(t + 1) * 128],
                                         projw_sb[:, c, :], start=(c == 0), stop=(c == 3))
                    x1 = x1p.tile([128, DIM], F32, name=f"x1_{gp}_{t}", tag=f"x1{t}")
                    nc.vector.tensor_tensor(x1[:], P[:], cur["xpb"][t][:], op=AL.add)
                    x1_t.append(x1)
                # LN2 -> x_hat2 fp16, batched transpose (sync ring)
                mv2 = tp.tile([128, 4, 2], F32, name=f"mv2_{gp}", tag="mv2")
                for t in range(4):
                    stats = tp.tile([128, 6], F32, name=f"st2_{gp}_{t}", tag=f"st2{t}")
                    nc.vector.bn_stats(stats[:], x1_t[t][:])
                    nc.vector.bn_aggr(mv2[:, t:t + 1, :], stats[:])
                std2 = tp.tile([128, 4], F32, name=f"sd2_{gp}", tag="sd2")
                nc.scalar.activation(std2[:], mv2[:, :, 1:2], AF.Sqrt, bias=eps_t[:])
                rs2 = tp.tile([128, 4], F32, name=f"rs2_{gp}", tag="rs2")
                nc.vector.reciprocal(rs2[:], std2[:])
                h2T = h2tp.tile([128, 4, GRP], F16, name=f"h2T_{gp}", tag="h2T")
                for t in range(4):
                    xh2 = xhp.tile([128, DIM], F16, name=f"xh2_{gp}_{t}", tag=f"xh2{t}")
                    nc.vector.tensor_scalar(xh2[:], x1_t[t][:], mv2[:, t:t + 1, 0:1], rs2[:, t:t + 1],
                                            op0=AL.subtract, op1=AL.mult)
                    nc.sync.dma_start_transpose(h2T[:, :, t * 128:(t + 1) * 128], xh2[:])
                # MLP1 + gelu (feature-major) fp16
                gel = gelp.tile([128, 16, GRP], F16, name=f"gel_{gp}", tag="gel")
                for f in range(16):
                    P = ps_g.tile([128, GRP], F32, name=f"Pm1_{gp}_{f}", tag="gemm")
                    for c in range(4):
                        nc.tensor.matmul(P[:], w1_sb[:, c, f * 128:(f + 1) * 128],
                                         h2T[:, c, :], start=(c == 0), stop=(c == 3))
                    nc.scalar.activation(gel[:, f, :], P[:], AF.Gelu, bias=b1_sb[:, f:f + 1])
                # MLP2 + bias + residual -> out (token-major)
                for t in range(4):
                    P = ps_g.tile([128, DIM], F32, name=f"Pm2_{gp}_{t}", tag="gemm")
                    for f in range(16):
                        nc.tensor.matmul(P[:], gel[:, f, t * 128:(t + 1) * 128],
                                         w2_sb[:, f, :], start=(f == 0), stop=(f == 15))
                    x1b = tp.tile([128, DIM], F32, name=f"x1b_{gp}_{t}", tag="x1b")
                    nc.gpsimd.tensor_tensor(x1b[:], x1_t[t][:], b2_bc[:], op=AL.add)
                    ot = op.tile([128, DIM], F32, name=f"o_{gp}_{t}", tag=f"o{t % 2}")
                    nc.vector.tensor_tensor(ot[:], P[:], x1b[:], op=AL.add)
                    nc.sync.dma_start(out_d[(gp * 4 + t) * 128:(gp * 4 + t + 1) * 128, :], ot[:])

            stages[0] = qkv_ln(0)
            for g in range(n_groups):
                qkv_mm(g)
                if g + 1 < n_groups:
                    stages[g + 1] = qkv_ln(g + 1)
                if g >= 1:
                    a16 = attn_stage(g - 1)
                    proj_mlp_stage(g - 1, a16)
            a16 = attn_stage(n_groups - 1)
            proj_mlp_stage(n_groups - 1, a16)

    nc.compile()
    return nc


_cache = {}


def _get_nc(n_tok):
    if n_tok not in _cache:
        _cache[n_tok] = build(n_tok)
    return _cache[n_tok]


def _prep_in_maps(inputs):
    return _prep(**inputs)


def _w16(w, chunks):
    """[K, M] f32 -> [128, K//128, M] fp16."""
    K, M = w.shape
    assert K == 128 * chunks
    return np.ascontiguousarray(
        w.astype(np.float16).reshape(chunks, 128, M).transpose(1, 0, 2))


def _prep(x, t_emb, ln1_g, ln1_b, qkv_w, qkv_b, proj_w, proj_b,
          ln2_g, ln2_b, mlp_w1, mlp_b1, mlp_w2, mlp_b2, time_w, time_b):
    x = np.asarray(x, dtype=np.float32)
    t_emb = np.asarray(t_emb, np.float32)
    # host: modulation rows (tiny), fold ln1 gamma/beta
    s = t_emb / (1.0 + np.exp(-t_emb))           # silu
    ss = s @ np.asarray(time_w, np.float32) + np.asarray(time_b, np.float32)
    scale, shift = ss[:, :DIM], ss[:, DIM:]
    g1 = np.asarray(ln1_g, np.float32)
    be1 = np.asarray(ln1_b, np.float32)
    arow = g1[None, :] * (1.0 + scale)                      # [B, 512]
    crow = be1[None, :] * (1.0 + scale) + shift             # [B, 512]
    # fold ln2 gamma/beta into mlp_w1/b1
    g2 = np.asarray(ln2_g, np.float32)
    be2 = np.asarray(ln2_b, np.float32)
    w1f = np.asarray(mlp_w1, np.float32) * g2[:, None]
    b1f = be2 @ np.asarray(mlp_w1, np.float32) + np.asarray(mlp_b1, np.float32)

    qkvw16 = _w16(np.asarray(qkv_w, np.float32), 4)
    projw16 = _w16(np.asarray(proj_w, np.float32), 4)
    w116 = _w16(w1f, 4)
    w216 = _w16(np.asarray(mlp_w2, np.float32), 16)
    qkvb = np.asarray(qkv_b, np.float32)
    qkb = np.ascontiguousarray(qkvb[:2 * DIM])
    vb = np.ascontiguousarray(qkvb[2 * DIM:])
    projb = np.asarray(proj_b, np.float32)
    b2 = np.asarray(mlp_b2, np.float32)

    in_maps = []
    nb = x.shape[0]
    for b in range(nb):
        in_maps.append({
            "x": np.ascontiguousarray(x[b]),
            "arow": np.ascontiguousarray(arow[b]),
            "crow": np.ascontiguousarray(crow[b]),
            "qkvw": qkvw16, "qkb": qkb, "vb": vb,
            "projw": projw16, "projb": projb,
            "w1": w116, "b1": b1f, "w2": w216, "b2": b2,
        })
    return in_maps


def kernel(**inputs):
    in_maps = _prep_in_maps(inputs)
    n_tok = in_maps[0]["x"].shape[0]
    nc = _get_nc(n_tok)
    nb = len(in_maps)
    res = bass_utils.run_bass_kernel_spmd(nc, in_maps, core_ids=list(range(nb)))
    out = np.stack([res.results[b]["out"] for b in range(nb)], axis=0)
    return out


# revision 30
# speedup vs baseline: 1.0705x; 1.0705x over previous
"""Trainium2 Bass kernel for a local-attention transformer block (v3, fp16).

Computes, per batch element (one NeuronCore each, 8 cores):
  ss = silu(t_emb) @ time_w + time_b ;  scale, shift = split(ss)
  y  = LN(x) * (1+scale) + shift                       (ln1 g/b host-folded)
  q,k,v = y @ qkv_w + qkv_b  (heads=8, d=64)
  attn: each 128-token window attends to [prev|cur|next] windows
  x1 = x + attn @ proj_w + proj_b
  out = x1 + gelu(LN2(x1) @ w1 + b1') @ w2 + b2        (ln2 g/b folded into w1/b1)

v3 strategy (evolved from the 2.18 ms fp16 baseline):
  - All GEMMs fp16 (measured: fp16=bf16=fp8 all stream 216 ns per N=512 matmul;
    DoubleRow's 256-col LDWEIGHTS doesn't background-load, so fp8 gains nothing).
    Weights stored [128, n_chunks, out] fp16, activations transposed to
    [128, n_chunks, 512] fp16 chunk tiles.
  - Attention key-block-major: per (head, key block j) ONE sim matmul of
    N<=384 (q windows j-1..j+1, keys on partitions), exp into an E tile
    reused by 3 AV windows; AV accumulates [65, 4win, 128] PSUM per head
    (ones column folded into v_aug produces softmax denominators).
  - LN transposes x_hat fp16 via ONE batched DMA transpose per token tile
    ([128,512] -> [128,4,128], same 1.2us as a 128x128 transpose); modulate
    fused into a per-chunk tensor_scalar on the transposed side.
  - ACT engine runs ONLY Exp/Gelu/Sqrt (no Identity copies, no DMA issue):
    PSUM evacuations + bias adds on DVE (cross-partition DVE copies verified),
    per-head softmax reciprocal on DVE directly from PSUM row 64.
  - All DMA on the sync ring (transposes batched); scalar ring unused so the
    ACT queue stays clean.
"""

import numpy as np
from contextlib import ExitStack

import concourse.bass as bass
import concourse.tile as tile
from concourse import bacc, mybir
from concourse import bass_utils

F32 = mybir.dt.float32
F16 = mybir.dt.float16
BF16 = mybir.dt.bfloat16
AF = mybir.ActivationFunctionType
AL = mybir.AluOpType

DIM = 512
HEADS = 8
HD = 64
FF = 2048
WIN = 128
B = 8
NTOK = 8192
EPS = 1e-5
GRP = 512  # tokens per group (4 windows)
SIMSCALE = float(HD) ** -0.5


def _col_view(dram_ap, offset, ncol):
    """AP reading dram vector [128*ncol] as [128, ncol] feature-major columns."""
    return bass.AP(tensor=dram_ap.tensor, offset=offset, ap=[[1, 128], [128, ncol]])


def _bcast_row(dram_ap, offset, n):
    """AP reading dram vector [n] broadcast across 128 partitions."""
    return bass.AP(tensor=dram_ap.tensor, offset=offset, ap=[[0, 128], [1, n]])


def build(n_tok=NTOK):
    n_groups = n_tok // GRP
    nW = n_tok // WIN
    nc = bacc.Bacc("TRN2", target_bir_lowering=False, debug=False)

    x_d = nc.dram_tensor("x", [n_tok, DIM], F32, kind="ExternalInput")
    arow_d = nc.dram_tensor("arow", [DIM], F32, kind="ExternalInput")
    crow_d = nc.dram_tensor("crow", [DIM], F32, kind="ExternalInput")
    qkvw_d = nc.dram_tensor("qkvw", [128, 4, 3 * DIM], F16, kind="ExternalInput")
    qkb_d = nc.dram_tensor("qkb", [2 * DIM], F32, kind="ExternalInput")
    vb_d = nc.dram_tensor("vb", [DIM], F32, kind="ExternalInput")
    projw_d = nc.dram_tensor("projw", [128, 4, DIM], F16, kind="ExternalInput")
    projb_d = nc.dram_tensor("projb", [DIM], F32, kind="ExternalInput")
    w1_d = nc.dram_tensor("w1", [128, 4, FF], F16, kind="ExternalInput")
    b1_d = nc.dram_tensor("b1", [FF], F32, kind="ExternalInput")
    w2_d = nc.dram_tensor("w2", [128, 16, DIM], F16, kind="ExternalInput")
    b2_d = nc.dram_tensor("b2", [DIM], F32, kind="ExternalInput")
    out_d = nc.dram_tensor("out", [n_tok, DIM], F32, kind="ExternalOutput")

    with tile.TileContext(nc) as tc:
        with ExitStack() as ctx:
            consts = ctx.enter_context(tc.tile_pool(name="consts", bufs=1))
            xp = ctx.enter_context(tc.tile_pool(name="xp", bufs=2))
            xpbp = ctx.enter_context(tc.tile_pool(name="xpbp", bufs=1))
            xhp = ctx.enter_context(tc.tile_pool(name="xhp", bufs=1))
            ytp = ctx.enter_context(tc.tile_pool(name="ytp", bufs=2))
            qp = ctx.enter_context(tc.tile_pool(name="qp", bufs=2))
            kp = ctx.enter_context(tc.tile_pool(name="kp", bufs=2))
            vp = ctx.enter_context(tc.tile_pool(name="vp", bufs=2))
            ep = ctx.enter_context(tc.tile_pool(name="ep", bufs=1))
            astp = ctx.enter_context(tc.tile_pool(name="astp", bufs=1))
            a16p = ctx.enter_context(tc.tile_pool(name="a16p", bufs=1))
            x1p = ctx.enter_context(tc.tile_pool(name="x1p", bufs=1))
            h2tp = ctx.enter_context(tc.tile_pool(name="h2tp", bufs=1))
            gelp = ctx.enter_context(tc.tile_pool(name="gelp", bufs=1))
            op = ctx.enter_context(tc.tile_pool(name="op", bufs=2))
            sp = ctx.enter_context(tc.tile_pool(name="sp", bufs=2))
            rp = ctx.enter_context(tc.tile_pool(name="rp", bufs=1))
            tp = ctx.enter_context(tc.tile_pool(name="tp", bufs=2))
            dp = ctx.enter_context(tc.tile_pool(name="dp", bufs=2, space="DRAM"))
            ps_g = ctx.enter_context(tc.tile_pool(name="ps_g", bufs=3, space="PSUM"))
            ps_s = ctx.enter_context(tc.tile_pool(name="ps_s", bufs=3, space="PSUM"))
            ps_a = ctx.enter_context(tc.tile_pool(name="ps_a", bufs=2, space="PSUM"))

            # ---- constants ----
            qkvw_sb = consts.tile([128, 4, 3 * DIM], F16, name="qkvw_sb")
            nc.sync.dma_start(qkvw_sb[:], qkvw_d[:, :, :])
            projw_sb = consts.tile([128, 4, DIM], F16, name="projw_sb")
            nc.sync.dma_start(projw_sb[:], projw_d[:, :, :])
            w1_sb = consts.tile([128, 4, FF], F16, name="w1_sb")
            nc.sync.dma_start(w1_sb[:], w1_d[:, :, :])
            w2_sb = consts.tile([128, 16, DIM], F16, name="w2_sb")
            nc.sync.dma_start(w2_sb[:], w2_d[:, :, :])

            arow_col = consts.tile([128, 4], F32, name="arow_col")
            nc.sync.dma_start(arow_col[:], _col_view(arow_d.ap(), 0, 4))
            crow_col = consts.tile([128, 4], F32, name="crow_col")
            nc.sync.dma_start(crow_col[:], _col_view(crow_d.ap(), 0, 4))
            qkb_sb = consts.tile([128, 8], F32, name="qkb_sb")
            nc.sync.dma_start(qkb_sb[:], _col_view(qkb_d.ap(), 0, 8))
            b1_sb = consts.tile([128, 16], F32, name="b1_sb")
            nc.sync.dma_start(b1_sb[:], _col_view(b1_d.ap(), 0, 16))
            vb_bc = consts.tile([128, DIM], F32, name="vb_bc")
            nc.sync.dma_start(vb_bc[:], _bcast_row(vb_d.ap(), 0, DIM))
            projb_bc = consts.tile([128, DIM], F32, name="projb_bc")
            nc.sync.dma_start(projb_bc[:], _bcast_row(projb_d.ap(), 0, DIM))
            b2_bc = consts.tile([128, DIM], F32, name="b2_bc")
            nc.sync.dma_start(b2_bc[:], _bcast_row(b2_d.ap(), 0, DIM))
            eps_t = consts.tile([128, 1], F32, name="eps_t")
            nc.vector.memset(eps_t[:], EPS)

            stages = {}   # g -> dict of tiles
            e_tiles = {}  # (h, j) -> E tile

            def qkv_ln(g):
                """x load + LN1 + transpose + modulate -> y16 (no matmuls).

                Emitted one group early so its ACT sqrt isn't queued behind
                the attention exps and PE always has QKV work ready."""
                st = {}
                xts, xpbs = [], []
                mv = tp.tile([128, 4, 2], F32, name=f"mv1_{g}", tag="mv1")
                for t in range(4):
                    xt = xp.tile([128, DIM], F32, name=f"x_{g}_{t}", tag=f"x{t}")
                    nc.sync.dma_start(xt[:], x_d[(g * 4 + t) * 128:(g * 4 + t + 1) * 128, :])
                    stats = tp.tile([128, 6], F32, name=f"st_{g}_{t}", tag=f"st{t}")
                    nc.vector.bn_stats(stats[:], xt[:])
                    nc.vector.bn_aggr(mv[:, t:t + 1, :], stats[:])
                    xts.append(xt)
                std = tp.tile([128, 4], F32, name=f"sd_{g}", tag="sd1")
                nc.scalar.activation(std[:], mv[:, :, 1:2], AF.Sqrt, bias=eps_t[:])
                rs = tp.tile([128, 4], F32, name=f"rs_{g}", tag="rs1")
                nc.vector.reciprocal(rs[:], std[:])
                xhT = ytp.tile([128, 4, GRP], F16, name=f"xhT_{g}", tag="xhT")
                for t in range(4):
                    xh = xhp.tile([128, DIM], F16, name=f"xh_{g}_{t}", tag=f"xh{t}")
                    nc.vector.tensor_scalar(xh[:], xts[t][:], mv[:, t:t + 1, 0:1], rs[:, t:t + 1],
                                            op0=AL.subtract, op1=AL.mult)
                    # x + projb precomputed so x tiles die early
                    xpb = xpbp.tile([128, DIM], F32, name=f"xpb_{g}_{t}", tag=f"xpb{t}")
                    nc.vector.tensor_tensor(xpb[:], xts[t][:], projb_bc[:], op=AL.add)
                    xpbs.append(xpb)
                    # one batched transpose per token tile (sync ring)
                    nc.sync.dma_start_transpose(xhT[:, :, t * 128:(t + 1) * 128], xh[:])
                st["xpb"] = xpbs
                # modulate per chunk (arow/crow are per-partition on transposed side)
                y16 = ytp.tile([128, 4, GRP], F16, name=f"y16_{g}", tag="y16")
                for c in range(4):
                    nc.vector.tensor_scalar(y16[:, c, :], xhT[:, c, :],
                                            arow_col[:, c:c + 1], crow_col[:, c:c + 1],
                                            op0=AL.mult, op1=AL.add)
                st["y16"] = y16
                return st

            def qkv_mm(g):
                st = stages[g]
                y16 = st["y16"]
                # Q,K: weight-stationary, out feature-major [128 f, 512 tok]
                q_t, k_t = [], []
                for m in range(8):
                    P = ps_g.tile([128, GRP], F32, name=f"Pqk_{g}_{m}", tag="gemm")
                    for c in range(4):
                        nc.tensor.matmul(P[:], qkvw_sb[:, c, m * 128:(m + 1) * 128],
                                         y16[:, c, :], start=(c == 0), stop=(c == 3))
                    pool = qp if m < 4 else kp
                    nm = f"q_{g}_{m}" if m < 4 else f"k_{g}_{m-4}"
                    tg = f"q{m}" if m < 4 else f"k{m-4}"
                    sb = pool.tile([128, GRP], F16, name=nm, tag=tg)
                    nc.vector.tensor_scalar_add(sb[:], P[:], qkb_sb[:, m:m + 1])
                    (q_t if m < 4 else k_t).append(sb)
                st["q"], st["k"] = q_t, k_t
                # V: activation-stationary, out token-major + ones col
                v_t = []
                for t in range(4):
                    P = ps_g.tile([128, DIM], F32, name=f"Pv_{g}_{t}", tag="gemm")
                    for c in range(4):
                        nc.tensor.matmul(P[:], y16[:, c, t * 128:(t + 1) * 128],
                                         qkvw_sb[:, c, 2 * DIM:3 * DIM],
                                         start=(c == 0), stop=(c == 3))
                    vt = vp.tile([128, HEADS, HD + 1], F16, name=f"v_{g}_{t}", tag=f"v{t}")
                    nc.vector.memset(vt[:, :, HD:HD + 1], 1.0)
                    nc.vector.tensor_tensor(
                        vt[:, :, 0:HD],
                        P[:].rearrange("p (h d) -> p h d", h=HEADS),
                        vb_bc[:].rearrange("p (h d) -> p h d", h=HEADS),
                        op=AL.add)
                    v_t.append(vt)
                st["v"] = v_t

            def attn_stage(gp):
                # key blocks computed this phase
                j_list = [j for j in range(4 * gp + 1, 4 * gp + 5) if j < nW]
                if gp == 0:
                    j_list = [0] + j_list
                attn_f16 = {}
                sums_dr = dp.tile([8, GRP], F32, name=f"sums_{gp}", tag="sums")
                for hp in range(4):
                    heads = (2 * hp, 2 * hp + 1)
                    af = astp.tile([128, GRP], F16, name=f"af_{gp}_{hp}", tag=f"af{hp}")
                    attn_f16[hp] = af
                    P_av = {}
                    for h in heads:
                        P_av[h] = ps_a.tile([65, 4, 128], F32, name=f"Pav_{gp}_{h}", tag="av")
                    for j in j_list:
                        gj, sj = divmod(j, 4)
                        qlo = max(0, j - 1)
                        qhi = min(nW - 1, j + 1)
                        ncols = (qhi - qlo + 1) * WIN
                        for h in heads:
                            off = (h % 2) * 64
                            P_sim = ps_s.tile([128, 384], F32, name=f"Ps_{gp}_{h}_{j}", tag="sim")
                            # q columns may span two group tiles -> split segments
                            w0 = qlo
                            while w0 <= qhi:
                                gq = w0 // 4
                                wend = min(qhi, gq * 4 + 3)
                                c0 = (w0 % 4) * WIN
                                c1 = (wend % 4 + 1) * WIN
                                dst0 = (w0 - qlo) * WIN
                                nc.tensor.matmul(
                                    P_sim[:, dst0:dst0 + (c1 - c0)],
                                    stages[gj]["k"][hp][off:off + 64, sj * 128:(sj + 1) * 128],
                                    stages[gq]["q"][hp][off:off + 64, c0:c1],
                                    start=True, stop=True)
                                w0 = wend + 1
                            E = ep.tile([128, 384], F16, name=f"E_{gp}_{h}_{j}",
                                        tag=f"E{h}_{j % 3}")
                            nc.scalar.activation(E[:, 0:ncols], P_sim[:, 0:ncols],
                                                 AF.Exp, scale=SIMSCALE)
                            e_tiles[(h, j)] = E
                        # AV for completed windows
                        av_ws = []
                        if 4 * gp <= j - 1 <= 4 * gp + 3:
                            av_ws.append(j - 1)
                        if j == j_list[-1] and j == nW - 1:
                            av_ws.append(nW - 1)
                        for w in av_ws:
                            jjs = [jj for jj in (w - 1, w, w + 1) if 0 <= jj < nW]
                            for h in heads:
                                for ji, jj in enumerate(jjs):
                                    gjj, sjj = divmod(jj, 4)
                                    colofs = (w - max(0, jj - 1)) * WIN
                                    nc.tensor.matmul(
                                        P_av[h][:, w % 4, :],
                                        stages[gjj]["v"][sjj][:, h, :],
                                        e_tiles[(h, jj)][:, colofs:colofs + WIN],
                                        start=(ji == 0), stop=(ji == len(jjs) - 1))
                    # evacuate PSUM: unnormalized attn (DVE when aligned) + sums
                    for h in heads:
                        off = (h % 2) * 64
                        src = P_av[h][0:64, :, :].rearrange("p a b -> p (a b)")
                        if h % 2 == 0:
                            nc.vector.tensor_copy(af[0:64, :], src)
                        else:
                            nc.scalar.activation(af[64:128, :], src, AF.Identity)
                        ss = sp.tile([1, GRP], F32, name=f"ss_{gp}_{h}", tag="ss")
                        nc.scalar.activation(ss[:],
                                             P_av[h][64:65, :, :].rearrange("p a b -> p (a b)"),
                                             AF.Identity)
                        nc.sync.dma_start(sums_dr[h:h + 1, :], ss[:])
                # one batched reciprocal + broadcast (DRAM round trip), normalize -> fp16
                sums_sb = sp.tile([8, GRP], F32, name=f"sums_sb_{gp}", tag="sums_sb")
                nc.sync.dma_start(sums_sb[:], sums_dr[:])
                r_sb = sp.tile([8, GRP], F32, name=f"r_sb_{gp}", tag="r_sb")
                nc.vector.reciprocal(r_sb[:], sums_sb[:])
                r_dr = dp.tile([8, GRP], F32, name=f"r_dr_{gp}", tag="rd")
                nc.sync.dma_start(r_dr[:], r_sb[:])
                a16 = a16p.tile([128, 4, GRP], F16, name=f"a16_{gp}", tag="a16")
                for hp in range(4):
                    rbc = rp.tile([128, GRP], F32, name=f"rbc_{gp}_{hp}", tag=f"rbc{hp % 2}")
                    src = bass.AP(tensor=r_dr[:].tensor, offset=r_dr[:].offset + 2 * hp * GRP,
                                  ap=[[GRP, 2], [0, 64], [1, GRP]])
                    nc.sync.dma_start(rbc[:], src)
                    nc.vector.tensor_tensor(a16[:, hp, :], attn_f16[hp][:], rbc[:], op=AL.mult)
                return a16

            def proj_mlp_stage(gp, a16):
                cur = stages[gp]
                # proj + residual -> x1 (token-major f32)
                x1_t = []
                for t in range(4):
                    P = ps_g.tile([128, DIM], F32, name=f"Ppr_{gp}_{t}", tag="gemm")
                    for c in range(4):
                        nc.tensor.matmul(P[:], a16[:, c, t * 128:(t + 1) * 128],
                                         projw_sb[:, c, :], start=(c == 0), stop=(c == 3))
                    x1 = x1p.tile([128, DIM], F32, name=f"x1_{gp}_{t}", tag=f"x1{t}")
                    nc.vector.tensor_tensor(x1[:], P[:], cur["xpb"][t][:], op=AL.add)
                    x1_t.append(x1)
                # LN2 -> x_hat2 fp16, batched transpose (sync ring)
                mv2 = tp.tile([128, 4, 2], F32, name=f"mv2_{gp}", tag="mv2")
                for t in range(4):
                    stats = tp.tile([128, 6], F32, name=f"st2_{gp}_{t}", tag=f"st2{t}")
                    nc.vector.bn_stats(stats[:], x1_t[t][:])
                    nc.vector.bn_aggr(mv2[:, t:t + 1, :], stats[:])
                std2 = tp.tile([128, 4], F32, name=f"sd2_{gp}", tag="sd2")
                nc.scalar.activation(std2[:], mv2[:, :, 1:2], AF.Sqrt, bias=eps_t[:])
                rs2 = tp.tile([128, 4], F32, name=f"rs2_{gp}", tag="rs2")
                nc.vector.reciprocal(rs2[:], std2[:])
                h2T = h2tp.tile([128, 4, GRP], F16, name=f"h2T_{gp}", tag="h2T")
                for t in range(4):
                    xh2 = xhp.tile([128, DIM], F16, name=f"xh2_{gp}_{t}", tag=f"xh2{t}")
                    nc.vector.tensor_scalar(xh2[:], x1_t[t][:], mv2[:, t:t + 1, 0:1], rs2[:, t:t + 1],
                                            op0=AL.subtract, op1=AL.mult)
                    nc.sync.dma_start_transpose(h2T[:, :, t * 128:(t + 1) * 128], xh2[:])
                # MLP1 + gelu (feature-major) fp16
                gel = gelp.tile([128, 16, GRP], F16, name=f"gel_{gp}", tag="gel")
                for f in range(16):
                    P = ps_g.tile([128, GRP], F32, name=f"Pm1_{gp}_{f}", tag="gemm")
                    for c in range(4):
                        nc.tensor.matmul(P[:], w1_sb[:, c, f * 128:(f + 1) * 128],
                                         h2T[:, c, :], start=(c == 0), stop=(c == 3))
                    nc.scalar.activation(gel[:, f, :], P[:], AF.Gelu, bias=b1_sb[:, f:f + 1])
                # MLP2 + bias + residual -> out (token-major)
                for t in range(4):
                    P = ps_g.tile([128, DIM], F32, name=f"Pm2_{gp}_{t}", tag="gemm")
                    for f in range(16):
                        nc.tensor.matmul(P[:], gel[:, f, t * 128:(t + 1) * 128],
                                         w2_sb[:, f, :], start=(f == 0), stop=(f == 15))
                    x1b = tp.tile([128, DIM], F32, name=f"x1b_{gp}_{t}", tag="x1b")
                    nc.vector.tensor_tensor(x1b[:], x1_t[t][:], b2_bc[:], op=AL.add)
                    ot = op.tile([128, DIM], F32, name=f"o_{gp}_{t}", tag="o")
                    nc.vector.tensor_tensor(ot[:], P[:], x1b[:], op=AL.add)
                    nc.sync.dma_start(out_d[(gp * 4 + t) * 128:(gp * 4 + t + 1) * 128, :], ot[:])

            stages[0] = qkv_ln(0)
            for g in range(n_groups):
                qkv_mm(g)
                if g + 1 < n_groups:
                    stages[g + 1] = qkv_ln(g + 1)
                if g >= 1:
                    a16 = attn_stage(g - 1)
                    proj_mlp_stage(g - 1, a16)
            a16 = attn_stage(n_groups - 1)
            proj_mlp_stage(n_groups - 1, a16)

    nc.compile()
    return nc


_cache = {}


def _get_nc(n_tok):
    if n_tok not in _cache:
        _cache[n_tok] = build(n_tok)
    return _cache[n_tok]


def _prep_in_maps(inputs):
    return _prep(**inputs)


def _w16(w, chunks):
    """[K, M] f32 -> [128, K//128, M] fp16."""
    K, M = w.shape
    assert K == 128 * chunks
    return np.ascontiguousarray(
        w.astype(np.float16).reshape(chunks, 128, M).transpose(1, 0, 2))


def _prep(x, t_emb, ln1_g, ln1_b, qkv_w, qkv_b, proj_w, proj_b,
          ln2_g, ln2_b, mlp_w1, mlp_b1, mlp_w2, mlp_b2, time_w, time_b):
    x = np.asarray(x, dtype=np.float32)
    t_emb = np.asarray(t_emb, np.float32)
    # host: modulation rows (tiny), fold ln1 gamma/beta
    s = t_emb / (1.0 + np.exp(-t_emb))           # silu
    ss = s @ np.asarray(time_w, np.float32) + np.asarray(time_b, np.float32)
    scale, shift = ss[:, :DIM], ss[:, DIM:]
    g1 = np.asarray(ln1_g, np.float32)
    be1 = np.asarray(ln1_b, np.float32)
    arow = g1[None, :] * (1.0 + scale)                      # [B, 512]
    crow = be1[None, :] * (1.0 + scale) + shift             # [B, 512]
    # fold ln2 gamma/beta into mlp_w1/b1
    g2 = np.asarray(ln2_g, np.float32)
    be2 = np.asarray(ln2_b, np.float32)
    w1f = np.asarray(mlp_w1, np.float32) * g2[:, None]
    b1f = be2 @ np.asarray(mlp_w1, np.float32) + np.asarray(mlp_b1, np.float32)

    qkvw16 = _w16(np.asarray(qkv_w, np.float32), 4)
    projw16 = _w16(np.asarray(proj_w, np.float32), 4)
    w116 = _w16(w1f, 4)
    w216 = _w16(np.asarray(mlp_w2, np.float32), 16)
    qkvb = np.asarray(qkv_b, np.float32)
    qkb = np.ascontiguousarray(qkvb[:2 * DIM])
    vb = np.ascontiguousarray(qkvb[2 * DIM:])
    projb = np.asarray(proj_b, np.float32)
    b2 = np.asarray(mlp_b2, np.float32)

    in_maps = []
    nb = x.shape[0]
    for b in range(nb):
        in_maps.append({
            "x": np.ascontiguousarray(x[b]),
            "arow": np.ascontiguousarray(arow[b]),
            "crow": np.ascontiguousarray(crow[b]),
            "qkvw": qkvw16, "qkb": qkb, "vb": vb,
            "projw": projw16, "projb": projb,
            "w1": w116, "b1": b1f, "w2": w216, "b2": b2,
        })
    return in_maps


def kernel(**inputs):
    in_maps = _prep_in_maps(inputs)
    n_tok = in_maps[0]["x"].shape[0]
    nc = _get_nc(n_tok)
    nb = len(in_maps)
    res = bass_utils.run_bass_kernel_spmd(nc, in_maps, core_ids=list(range(nb)))
    out = np.stack([res.results[b]["out"] for b in range(nb)], axis=0)
    return out


# revision 36
# speedup vs baseline: 1.1910x; 1.1125x over previous
"""Trainium2 Bass kernel for a local-attention transformer block (v3, fp16).

Computes, per batch element (one NeuronCore each, 8 cores):
  ss = silu(t_emb) @ time_w + time_b ;  scale, shift = split(ss)
  y  = LN(x) * (1+scale) + shift                       (ln1 g/b host-folded)
  q,k,v = y @ qkv_w + qkv_b  (heads=8, d=64)
  attn: each 128-token window attends to [prev|cur|next] windows
  x1 = x + attn @ proj_w + proj_b
  out = x1 + gelu(LN2(x1) @ w1 + b1') @ w2 + b2        (ln2 g/b folded into w1/b1)

v3 strategy (evolved from the 2.18 ms fp16 baseline):
  - All GEMMs fp16 (measured: fp16=bf16=fp8 all stream 216 ns per N=512 matmul;
    DoubleRow's 256-col LDWEIGHTS doesn't background-load, so fp8 gains nothing).
    Weights stored [128, n_chunks, out] fp16, activations transposed to
    [128, n_chunks, 512] fp16 chunk tiles.
  - Attention key-block-major: per (head, key block j) ONE sim matmul of
    N<=384 (q windows j-1..j+1, keys on partitions), exp into an E tile
    reused by 3 AV windows; AV accumulates [65, 4win, 128] PSUM per head
    (ones column folded into v_aug produces softmax denominators).
  - LN transposes x_hat fp16 via ONE batched DMA transpose per token tile
    ([128,512] -> [128,4,128], same 1.2us as a 128x128 transpose); modulate
    fused into a per-chunk tensor_scalar on the transposed side.
  - ACT engine runs ONLY Exp/Gelu/Sqrt (no Identity copies, no DMA issue):
    PSUM evacuations + bias adds on DVE (cross-partition DVE copies verified),
    per-head softmax reciprocal on DVE directly from PSUM row 64.
  - All DMA on the sync ring (transposes batched); scalar ring unused so the
    ACT queue stays clean.
"""

import numpy as np
from contextlib import ExitStack

import concourse.bass as bass
import concourse.tile as tile
from concourse import bacc, mybir
from concourse import bass_utils

F32 = mybir.dt.float32
F16 = mybir.dt.float16
BF16 = mybir.dt.bfloat16
AF = mybir.ActivationFunctionType
AL = mybir.AluOpType

DIM = 512
HEADS = 8
HD = 64
FF = 2048
WIN = 128
B = 8
NTOK = 8192
EPS = 1e-5
GRP = 512  # tokens per group (4 windows)
SIMSCALE = float(HD) ** -0.5


def _col_view(dram_ap, offset, ncol):
    """AP reading dram vector [128*ncol] as [128, ncol] feature-major columns."""
    return bass.AP(tensor=dram_ap.tensor, offset=offset, ap=[[1, 128], [128, ncol]])


def _bcast_row(dram_ap, offset, n):
    """AP reading dram vector [n] broadcast across 128 partitions."""
    return bass.AP(tensor=dram_ap.tensor, offset=offset, ap=[[0, 128], [1, n]])


def build(n_tok=NTOK):
    n_groups = n_tok // GRP
    nW = n_tok // WIN
    nc = bacc.Bacc("TRN2", target_bir_lowering=False, debug=False)

    x_d = nc.dram_tensor("x", [n_tok, DIM], F32, kind="ExternalInput")
    arow_d = nc.dram_tensor("arow", [DIM], F32, kind="ExternalInput")
    crow_d = nc.dram_tensor("crow", [DIM], F32, kind="ExternalInput")
    qkvw_d = nc.dram_tensor("qkvw", [128, 4, 3 * DIM], F16, kind="ExternalInput")
    qkb_d = nc.dram_tensor("qkb", [2 * DIM], F32, kind="ExternalInput")
    vb_d = nc.dram_tensor("vb", [DIM], F32, kind="ExternalInput")
    projw_d = nc.dram_tensor("projw", [128, 4, DIM], F16, kind="ExternalInput")
    projb_d = nc.dram_tensor("projb", [DIM], F32, kind="ExternalInput")
    w1_d = nc.dram_tensor("w1", [128, 4, FF], F16, kind="ExternalInput")
    b1_d = nc.dram_tensor("b1", [FF], F32, kind="ExternalInput")
    w2_d = nc.dram_tensor("w2", [128, 16, DIM], F16, kind="ExternalInput")
    b2_d = nc.dram_tensor("b2", [DIM], F32, kind="ExternalInput")
    out_d = nc.dram_tensor("out", [n_tok, DIM], F32, kind="ExternalOutput")

    with tile.TileContext(nc) as tc:
        with ExitStack() as ctx:
            consts = ctx.enter_context(tc.tile_pool(name="consts", bufs=1))
            xp = ctx.enter_context(tc.tile_pool(name="xp", bufs=2))
            xpbp = ctx.enter_context(tc.tile_pool(name="xpbp", bufs=3))
            xhp = ctx.enter_context(tc.tile_pool(name="xhp", bufs=1))
            ytp = ctx.enter_context(tc.tile_pool(name="ytp", bufs=2))
            qp = ctx.enter_context(tc.tile_pool(name="qp", bufs=2))
            kp = ctx.enter_context(tc.tile_pool(name="kp", bufs=2))
            vp = ctx.enter_context(tc.tile_pool(name="vp", bufs=2))
            ep = ctx.enter_context(tc.tile_pool(name="ep", bufs=1))
            astp = ctx.enter_context(tc.tile_pool(name="astp", bufs=1))
            a16p = ctx.enter_context(tc.tile_pool(name="a16p", bufs=1))
            x1p = ctx.enter_context(tc.tile_pool(name="x1p", bufs=1))
            h2tp = ctx.enter_context(tc.tile_pool(name="h2tp", bufs=1))
            gelp = ctx.enter_context(tc.tile_pool(name="gelp", bufs=1))
            op = ctx.enter_context(tc.tile_pool(name="op", bufs=2))
            sp = ctx.enter_context(tc.tile_pool(name="sp", bufs=2))
            rp = ctx.enter_context(tc.tile_pool(name="rp", bufs=1))
            tp = ctx.enter_context(tc.tile_pool(name="tp", bufs=2))
            dp = ctx.enter_context(tc.tile_pool(name="dp", bufs=2, space="DRAM"))
            ps_g = ctx.enter_context(tc.tile_pool(name="ps_g", bufs=3, space="PSUM"))
            ps_s = ctx.enter_context(tc.tile_pool(name="ps_s", bufs=3, space="PSUM"))
            ps_a = ctx.enter_context(tc.tile_pool(name="ps_a", bufs=2, space="PSUM"))

            # ---- constants ----
            qkvw_sb = consts.tile([128, 4, 3 * DIM], F16, name="qkvw_sb")
            nc.sync.dma_start(qkvw_sb[:], qkvw_d[:, :, :])
            projw_sb = consts.tile([128, 4, DIM], F16, name="projw_sb")
            nc.sync.dma_start(projw_sb[:], projw_d[:, :, :])
            w1_sb = consts.tile([128, 4, FF], F16, name="w1_sb")
            nc.sync.dma_start(w1_sb[:], w1_d[:, :, :])
            w2_sb = consts.tile([128, 16, DIM], F16, name="w2_sb")
            nc.sync.dma_start(w2_sb[:], w2_d[:, :, :])

            arow_col = consts.tile([128, 4], F32, name="arow_col")
            nc.sync.dma_start(arow_col[:], _col_view(arow_d.ap(), 0, 4))
            crow_col = consts.tile([128, 4], F32, name="crow_col")
            nc.sync.dma_start(crow_col[:], _col_view(crow_d.ap(), 0, 4))
            qkb_sb = consts.tile([128, 8], F32, name="qkb_sb")
            nc.sync.dma_start(qkb_sb[:], _col_view(qkb_d.ap(), 0, 8))
            b1_sb = consts.tile([128, 16], F32, name="b1_sb")
            nc.sync.dma_start(b1_sb[:], _col_view(b1_d.ap(), 0, 16))
            vb_bc = consts.tile([128, DIM], F32, name="vb_bc")
            nc.sync.dma_start(vb_bc[:], _bcast_row(vb_d.ap(), 0, DIM))
            projb_bc = consts.tile([128, DIM], F32, name="projb_bc")
            nc.sync.dma_start(projb_bc[:], _bcast_row(projb_d.ap(), 0, DIM))
            b2_bc = consts.tile([128, DIM], F32, name="b2_bc")
            nc.sync.dma_start(b2_bc[:], _bcast_row(b2_d.ap(), 0, DIM))
            eps_t = consts.tile([128, 1], F32, name="eps_t")
            nc.vector.memset(eps_t[:], EPS)

            stages = {}   # g -> dict of tiles
            e_tiles = {}  # (h, j) -> E tile

            def qkv_ln(g):
                """x load + LN1 + transpose + modulate -> y16 (no matmuls).

                Emitted one group early so its ACT sqrt isn't queued behind
                the attention exps and PE always has QKV work ready."""
                st = {}
                xts, xpbs = [], []
                mv = tp.tile([128, 4, 2], F32, name=f"mv1_{g}", tag="mv1")
                for t in range(4):
                    xt = xp.tile([128, DIM], F32, name=f"x_{g}_{t}", tag=f"x{t}")
                    nc.sync.dma_start(xt[:], x_d[(g * 4 + t) * 128:(g * 4 + t + 1) * 128, :])
                    stats = tp.tile([128, 6], F32, name=f"st_{g}_{t}", tag=f"st{t}")
                    nc.vector.bn_stats(stats[:], xt[:])
                    nc.vector.bn_aggr(mv[:, t:t + 1, :], stats[:])
                    xts.append(xt)
                std = tp.tile([128, 4], F32, name=f"sd_{g}", tag="sd1")
                nc.scalar.activation(std[:], mv[:, :, 1:2], AF.Sqrt, bias=eps_t[:])
                rs = tp.tile([128, 4], F32, name=f"rs_{g}", tag="rs1")
                nc.vector.reciprocal(rs[:], std[:])
                xhT = ytp.tile([128, 4, GRP], F16, name=f"xhT_{g}", tag="xhT")
                for t in range(4):
                    xh = xhp.tile([128, DIM], F16, name=f"xh_{g}_{t}", tag=f"xh{t}")
                    nc.vector.tensor_scalar(xh[:], xts[t][:], mv[:, t:t + 1, 0:1], rs[:, t:t + 1],
                                            op0=AL.subtract, op1=AL.mult)
                    # x + projb precomputed (fp16) so x tiles die early
                    xpb = xpbp.tile([128, DIM], F16, name=f"xpb_{g}_{t}", tag=f"xpb{t}")
                    nc.vector.tensor_tensor(xpb[:], xts[t][:], projb_bc[:], op=AL.add)
                    xpbs.append(xpb)
                    # one batched transpose per token tile (sync ring)
                    nc.sync.dma_start_transpose(xhT[:, :, t * 128:(t + 1) * 128], xh[:])
                st["xpb"] = xpbs
                # modulate per chunk (arow/crow are per-partition on transposed side)
                y16 = ytp.tile([128, 4, GRP], F16, name=f"y16_{g}", tag="y16")
                for c in range(4):
                    nc.vector.tensor_scalar(y16[:, c, :], xhT[:, c, :],
                                            arow_col[:, c:c + 1], crow_col[:, c:c + 1],
                                            op0=AL.mult, op1=AL.add)
                st["y16"] = y16
                st["q"] = [None] * 4
                st["k"] = [None] * 4
                st["v"] = [None] * 4
                return st

            def qkv_piece_qk(g, m):
                """One QK output chunk for group g (4 MMs + bias)."""
                st = stages[g]
                P = ps_g.tile([128, GRP], F32, name=f"Pqk_{g}_{m}", tag="gemm")
                for c in range(4):
                    nc.tensor.matmul(P[:], qkvw_sb[:, c, m * 128:(m + 1) * 128],
                                     st["y16"][:, c, :], start=(c == 0), stop=(c == 3))
                pool = qp if m < 4 else kp
                nm = f"q_{g}_{m}" if m < 4 else f"k_{g}_{m-4}"
                tg = f"q{m}" if m < 4 else f"k{m-4}"
                sb = pool.tile([128, GRP], F16, name=nm, tag=tg)
                nc.vector.tensor_scalar_add(sb[:], P[:], qkb_sb[:, m:m + 1])
                if m < 4:
                    st["q"][m] = sb
                else:
                    st["k"][m - 4] = sb

            def qkv_piece_v(g, t):
                """One V token tile for group g (4 MMs + bias + ones col)."""
                st = stages[g]
                P = ps_g.tile([128, DIM], F32, name=f"Pv_{g}_{t}", tag="gemm")
                for c in range(4):
                    nc.tensor.matmul(P[:], st["y16"][:, c, t * 128:(t + 1) * 128],
                                     qkvw_sb[:, c, 2 * DIM:3 * DIM],
                                     start=(c == 0), stop=(c == 3))
                vt = vp.tile([128, HEADS, HD + 1], F16, name=f"v_{g}_{t}", tag=f"v{t}")
                nc.vector.memset(vt[:, :, HD:HD + 1], 1.0)
                nc.vector.tensor_tensor(
                    vt[:, :, 0:HD],
                    P[:].rearrange("p (h d) -> p h d", h=HEADS),
                    vb_bc[:].rearrange("p (h d) -> p h d", h=HEADS),
                    op=AL.add)
                st["v"][t] = vt

            def make_pieces(g):
                """QKV matmul closures for group g, ordered so the chunks the
                next attention phase needs first are produced first."""
                ps = []
                for hp in range(4):
                    ps.append(lambda m=hp: qkv_piece_qk(g, m))
                    ps.append(lambda m=4 + hp: qkv_piece_qk(g, m))
                    ps.append(lambda t=hp: qkv_piece_v(g, t))
                return ps

            def attn_stage(gp, pieces):
                # key blocks computed this phase
                j_list = [j for j in range(4 * gp + 1, 4 * gp + 5) if j < nW]
                if gp == 0:
                    j_list = [0] + j_list
                attn_f16 = {}
                sums_dr = dp.tile([8, GRP], F32, name=f"sums_{gp}", tag="sums")
                r_dr = dp.tile([8, GRP], F32, name=f"r_dr_{gp}", tag="rd")
                a16 = a16p.tile([128, 4, GRP], F16, name=f"a16_{gp}", tag="a16")
                pieces = list(pieces)
                for hp in range(4):
                    heads = (2 * hp, 2 * hp + 1)
                    af = astp.tile([128, GRP], F16, name=f"af_{gp}_{hp}", tag=f"af{hp}")
                    attn_f16[hp] = af
                    P_av = {}
                    for h in heads:
                        P_av[h] = ps_a.tile([65, 4, 128], F32, name=f"Pav_{gp}_{h}", tag="av")
                    for j in j_list:
                        gj, sj = divmod(j, 4)
                        qlo = max(0, j - 1)
                        qhi = min(nW - 1, j + 1)
                        ncols = (qhi - qlo + 1) * WIN
                        for h in heads:
                            off = (h % 2) * 64
                            P_sim = ps_s.tile([128, 384], F32, name=f"Ps_{gp}_{h}_{j}", tag="sim")
                            # q columns may span two group tiles -> split segments
                            w0 = qlo
                            while w0 <= qhi:
                                gq = w0 // 4
                                wend = min(qhi, gq * 4 + 3)
                                c0 = (w0 % 4) * WIN
                                c1 = (wend % 4 + 1) * WIN
                                dst0 = (w0 - qlo) * WIN
                                nc.tensor.matmul(
                                    P_sim[:, dst0:dst0 + (c1 - c0)],
                                    stages[gj]["k"][hp][off:off + 64, sj * 128:(sj + 1) * 128],
                                    stages[gq]["q"][hp][off:off + 64, c0:c1],
                                    start=True, stop=True)
                                w0 = wend + 1
                            E = ep.tile([128, 384], F16, name=f"E_{gp}_{h}_{j}",
                                        tag=f"E{h}_{j % 3}")
                            nc.scalar.activation(E[:, 0:ncols], P_sim[:, 0:ncols],
                                                 AF.Exp, scale=SIMSCALE)
                            e_tiles[(h, j)] = E
                        # AV for completed windows
                        av_ws = []
                        if 4 * gp <= j - 1 <= 4 * gp + 3:
                            av_ws.append(j - 1)
                        if j == j_list[-1] and j == nW - 1:
                            av_ws.append(nW - 1)
                        for w in av_ws:
                            jjs = [jj for jj in (w - 1, w, w + 1) if 0 <= jj < nW]
                            for h in heads:
                                for ji, jj in enumerate(jjs):
                                    gjj, sjj = divmod(jj, 4)
                                    colofs = (w - max(0, jj - 1)) * WIN
                                    nc.tensor.matmul(
                                        P_av[h][:, w % 4, :],
                                        stages[gjj]["v"][sjj][:, h, :],
                                        e_tiles[(h, jj)][:, colofs:colofs + WIN],
                                        start=(ji == 0), stop=(ji == len(jjs) - 1))
                        # interleave one QKV piece of the next group between
                        # attention steps so the PE queue never blocks long
                        if pieces:
                            pieces.pop(0)()
                    # evacuate PSUM: unnormalized attn (DVE when aligned) + sums
                    for h in heads:
                        off = (h % 2) * 64
                        src = P_av[h][0:64, :, :].rearrange("p a b -> p (a b)")
                        if h % 2 == 0:
                            nc.vector.tensor_copy(af[0:64, :], src)
                        else:
                            nc.scalar.activation(af[64:128, :], src, AF.Identity)
                        ss = sp.tile([1, GRP], F32, name=f"ss_{gp}_{h}", tag="ss")
                        nc.scalar.activation(ss[:],
                                             P_av[h][64:65, :, :].rearrange("p a b -> p (a b)"),
                                             AF.Identity)
                        nc.sync.dma_start(sums_dr[h:h + 1, :], ss[:])
                    # per-pair reciprocal round trip ([128,8] view keeps all
                    # DVE lanes busy) + broadcast + normalize; only the last
                    # pair's chain trails the final AV matmul
                    seg = bass.AP(tensor=sums_dr[:].tensor,
                                  offset=sums_dr[:].offset + 2 * hp * GRP,
                                  ap=[[8, 128], [1, 8]])
                    sseg = sp.tile([128, 8], F32, name=f"sseg_{gp}_{hp}", tag="sseg")
                    nc.sync.dma_start(sseg[:], seg)
                    rseg = sp.tile([128, 8], F32, name=f"rseg_{gp}_{hp}", tag="rseg")
                    nc.vector.reciprocal(rseg[:], sseg[:])
                    rout = bass.AP(tensor=r_dr[:].tensor,
                                   offset=r_dr[:].offset + 2 * hp * GRP,
                                   ap=[[8, 128], [1, 8]])
                    nc.sync.dma_start(rout, rseg[:])
                    rbc = rp.tile([128, GRP], F32, name=f"rbc_{gp}_{hp}", tag=f"rbc{hp % 2}")
                    src = bass.AP(tensor=r_dr[:].tensor, offset=r_dr[:].offset + 2 * hp * GRP,
                                  ap=[[GRP, 2], [0, 64], [1, GRP]])
                    nc.sync.dma_start(rbc[:], src)
                    nc.vector.tensor_tensor(a16[:, hp, :], attn_f16[hp][:], rbc[:], op=AL.mult)
                for pc in pieces:
                    pc()
                return a16

            def proj_mlp_stage(gp, a16):
                cur = stages[gp]
                # proj + residual -> x1 (token-major f32)
                x1_t = []
                for t in range(4):
                    P = ps_g.tile([128, DIM], F32, name=f"Ppr_{gp}_{t}", tag="gemm")
                    for c in range(4):
                        nc.tensor.matmul(P[:], a16[:, c, t * 128:(t + 1) * 128],
                                         projw_sb[:, c, :], start=(c == 0), stop=(c == 3))
                    x1 = x1p.tile([128, DIM], F32, name=f"x1_{gp}_{t}", tag=f"x1{t}")
                    nc.vector.tensor_tensor(x1[:], P[:], cur["xpb"][t][:], op=AL.add)
                    x1_t.append(x1)
                # LN2 -> x_hat2 fp16, batched transpose (sync ring)
                mv2 = tp.tile([128, 4, 2], F32, name=f"mv2_{gp}", tag="mv2")
                for t in range(4):
                    stats = tp.tile([128, 6], F32, name=f"st2_{gp}_{t}", tag=f"st2{t}")
                    nc.vector.bn_stats(stats[:], x1_t[t][:])
                    nc.vector.bn_aggr(mv2[:, t:t + 1, :], stats[:])
                std2 = tp.tile([128, 4], F32, name=f"sd2_{gp}", tag="sd2")
                nc.scalar.activation(std2[:], mv2[:, :, 1:2], AF.Sqrt, bias=eps_t[:])
                rs2 = tp.tile([128, 4], F32, name=f"rs2_{gp}", tag="rs2")
                nc.vector.reciprocal(rs2[:], std2[:])
                h2T = h2tp.tile([128, 4, GRP], F16, name=f"h2T_{gp}", tag="h2T")
                for t in range(4):
                    xh2 = xhp.tile([128, DIM], F16, name=f"xh2_{gp}_{t}", tag=f"xh2{t}")
                    nc.vector.tensor_scalar(xh2[:], x1_t[t][:], mv2[:, t:t + 1, 0:1], rs2[:, t:t + 1],
                                            op0=AL.subtract, op1=AL.mult)
                    nc.sync.dma_start_transpose(h2T[:, :, t * 128:(t + 1) * 128], xh2[:])
                # MLP1 + gelu (feature-major) fp16
                gel = gelp.tile([128, 16, GRP], F16, name=f"gel_{gp}", tag="gel")
                for f in range(16):
                    P = ps_g.tile([128, GRP], F32, name=f"Pm1_{gp}_{f}", tag="gemm")
                    for c in range(4):
                        nc.tensor.matmul(P[:], w1_sb[:, c, f * 128:(f + 1) * 128],
                                         h2T[:, c, :], start=(c == 0), stop=(c == 3))
                    nc.scalar.activation(gel[:, f, :], P[:], AF.Gelu, bias=b1_sb[:, f:f + 1])
                # MLP2 + bias + residual -> out (token-major)
                for t in range(4):
                    P = ps_g.tile([128, DIM], F32, name=f"Pm2_{gp}_{t}", tag="gemm")
                    for f in range(16):
                        nc.tensor.matmul(P[:], gel[:, f, t * 128:(t + 1) * 128],
                                         w2_sb[:, f, :], start=(f == 0), stop=(f == 15))
                    x1b = tp.tile([128, DIM], F32, name=f"x1b_{gp}_{t}", tag="x1b")
                    nc.vector.tensor_tensor(x1b[:], x1_t[t][:], b2_bc[:], op=AL.add)
                    ot = op.tile([128, DIM], F32, name=f"o_{gp}_{t}", tag="o")
                    nc.vector.tensor_tensor(ot[:], P[:], x1b[:], op=AL.add)
                    nc.sync.dma_start(out_d[(gp * 4 + t) * 128:(gp * 4 + t + 1) * 128, :], ot[:])

            stages[0] = qkv_ln(0)
            if n_groups > 1:
                stages[1] = qkv_ln(1)
            for pc in make_pieces(0):
                pc()
            for gp in range(n_groups):
                pieces = make_pieces(gp + 1) if gp + 1 < n_groups else []
                a16 = attn_stage(gp, pieces)
                if gp + 2 < n_groups:
                    stages[gp + 2] = qkv_ln(gp + 2)
                proj_mlp_stage(gp, a16)

    nc.compile()
    return nc


_cache = {}


def _get_nc(n_tok):
    if n_tok not in _cache:
        _cache[n_tok] = build(n_tok)
    return _cache[n_tok]


def _prep_in_maps(inputs):
    return _prep(**inputs)


def _w16(w, chunks):
    """[K, M] f32 -> [128, K//128, M] fp16."""
    K, M = w.shape
    assert K == 128 * chunks
    return np.ascontiguousarray(
        w.astype(np.float16).reshape(chunks, 128, M).transpose(1, 0, 2))


def _prep(x, t_emb, ln1_g, ln1_b, qkv_w, qkv_b, proj_w, proj_b,
          ln2_g, ln2_b, mlp_w1, mlp_b1, mlp_w2, mlp_b2, time_w, time_b):
    x = np.asarray(x, dtype=np.float32)
    t_emb = np.asarray(t_emb, np.float32)
    # host: modulation rows (tiny), fold ln1 gamma/beta
    s = t_emb / (1.0 + np.exp(-t_emb))           # silu
    ss = s @ np.asarray(time_w, np.float32) + np.asarray(time_b, np.float32)
    scale, shift = ss[:, :DIM], ss[:, DIM:]
    g1 = np.asarray(ln1_g, np.float32)
    be1 = np.asarray(ln1_b, np.float32)
    arow = g1[None, :] * (1.0 + scale)                      # [B, 512]
    crow = be1[None, :] * (1.0 + scale) + shift             # [B, 512]
    # fold ln2 gamma/beta into mlp_w1/b1
    g2 = np.asarray(ln2_g, np.float32)
    be2 = np.asarray(ln2_b, np.float32)
    w1f = np.asarray(mlp_w1, np.float32) * g2[:, None]
    b1f = be2 @ np.asarray(mlp_w1, np.float32) + np.asarray(mlp_b1, np.float32)

    qkvw16 = _w16(np.asarray(qkv_w, np.float32), 4)
    projw16 = _w16(np.asarray(proj_w, np.float32), 4)
    w116 = _w16(w1f, 4)
    w216 = _w16(np.asarray(mlp_w2, np.float32), 16)
    qkvb = np.asarray(qkv_b, np.float32)
    qkb = np.ascontiguousarray(qkvb[:2 * DIM])
    vb = np.ascontiguousarray(qkvb[2 * DIM:])
    projb = np.asarray(proj_b, np.float32)
    b2 = np.asarray(mlp_b2, np.float32)

    in_maps = []
    nb = x.shape[0]
    for b in range(nb):
        in_maps.append({
            "x": np.ascontiguousarray(x[b]),
            "arow": np.ascontiguousarray(arow[b]),
            "crow": np.ascontiguousarray(crow[b]),
            "qkvw": qkvw16, "qkb": qkb, "vb": vb,
            "projw": projw16, "projb": projb,
            "w1": w116, "b1": b1f, "w2": w216, "b2": b2,
        })
    return in_maps


def kernel(**inputs):
    in_maps = _prep_in_maps(inputs)
    n_tok = in_maps[0]["x"].shape[0]
    nc = _get_nc(n_tok)
    nb = len(in_maps)
    res = bass_utils.run_bass_kernel_spmd(nc, in_maps, core_ids=list(range(nb)))
    out = np.stack([res.results[b]["out"] for b in range(nb)], axis=0)
    return out


# revision 39
# speedup vs baseline: 1.2966x; 1.0887x over previous
"""Trainium2 Bass kernel for a local-attention transformer block (v3, fp16).

Computes, per batch element (one NeuronCore each, 8 cores):
  ss = silu(t_emb) @ time_w + time_b ;  scale, shift = split(ss)
  y  = LN(x) * (1+scale) + shift                       (ln1 g/b host-folded)
  q,k,v = y @ qkv_w + qkv_b  (heads=8, d=64)
  attn: each 128-token window attends to [prev|cur|next] windows
  x1 = x + attn @ proj_w + proj_b
  out = x1 + gelu(LN2(x1) @ w1 + b1') @ w2 + b2        (ln2 g/b folded into w1/b1)

v3 strategy (evolved from the 2.18 ms fp16 baseline):
  - All GEMMs fp16 (measured: fp16=bf16=fp8 all stream 216 ns per N=512 matmul;
    DoubleRow's 256-col LDWEIGHTS doesn't background-load, so fp8 gains nothing).
    Weights stored [128, n_chunks, out] fp16, activations transposed to
    [128, n_chunks, 512] fp16 chunk tiles.
  - Attention key-block-major: per (head, key block j) ONE sim matmul of
    N<=384 (q windows j-1..j+1, keys on partitions), exp into an E tile
    reused by 3 AV windows; AV accumulates [65, 4win, 128] PSUM per head
    (ones column folded into v_aug produces softmax denominators).
  - LN transposes x_hat fp16 via ONE batched DMA transpose per token tile
    ([128,512] -> [128,4,128], same 1.2us as a 128x128 transpose); modulate
    fused into a per-chunk tensor_scalar on the transposed side.
  - ACT engine runs ONLY Exp/Gelu/Sqrt (no Identity copies, no DMA issue):
    PSUM evacuations + bias adds on DVE (cross-partition DVE copies verified),
    per-head softmax reciprocal on DVE directly from PSUM row 64.
  - All DMA on the sync ring (transposes batched); scalar ring unused so the
    ACT queue stays clean.
"""

import numpy as np
from contextlib import ExitStack

import concourse.bass as bass
import concourse.tile as tile
from concourse import bacc, mybir
from concourse import bass_utils

F32 = mybir.dt.float32
F16 = mybir.dt.float16
BF16 = mybir.dt.bfloat16
AF = mybir.ActivationFunctionType
AL = mybir.AluOpType

DIM = 512
HEADS = 8
HD = 64
FF = 2048
WIN = 128
B = 8
NTOK = 8192
EPS = 1e-5
GRP = 512  # tokens per group (4 windows)
SIMSCALE = float(HD) ** -0.5


def _col_view(dram_ap, offset, ncol):
    """AP reading dram vector [128*ncol] as [128, ncol] feature-major columns."""
    return bass.AP(tensor=dram_ap.tensor, offset=offset, ap=[[1, 128], [128, ncol]])


def _bcast_row(dram_ap, offset, n):
    """AP reading dram vector [n] broadcast across 128 partitions."""
    return bass.AP(tensor=dram_ap.tensor, offset=offset, ap=[[0, 128], [1, n]])


def build(n_tok=NTOK):
    n_groups = n_tok // GRP
    nW = n_tok // WIN
    nc = bacc.Bacc("TRN2", target_bir_lowering=False, debug=False)

    x_d = nc.dram_tensor("x", [n_tok, DIM], F32, kind="ExternalInput")
    arow_d = nc.dram_tensor("arow", [DIM], F32, kind="ExternalInput")
    crow_d = nc.dram_tensor("crow", [DIM], F32, kind="ExternalInput")
    qkvw_d = nc.dram_tensor("qkvw", [128, 4, 3 * DIM], F16, kind="ExternalInput")
    qkb_d = nc.dram_tensor("qkb", [2 * DIM], F32, kind="ExternalInput")
    vb_d = nc.dram_tensor("vb", [DIM], F32, kind="ExternalInput")
    projw_d = nc.dram_tensor("projw", [128, 4, DIM], F16, kind="ExternalInput")
    projb_d = nc.dram_tensor("projb", [DIM], F32, kind="ExternalInput")
    w1_d = nc.dram_tensor("w1", [128, 4, FF], F16, kind="ExternalInput")
    b1_d = nc.dram_tensor("b1", [FF], F32, kind="ExternalInput")
    w2_d = nc.dram_tensor("w2", [128, 16, DIM], F16, kind="ExternalInput")
    b2_d = nc.dram_tensor("b2", [DIM], F32, kind="ExternalInput")
    out_d = nc.dram_tensor("out", [n_tok, DIM], F32, kind="ExternalOutput")

    with tile.TileContext(nc) as tc:
        with ExitStack() as ctx:
            consts = ctx.enter_context(tc.tile_pool(name="consts", bufs=1))
            xp = ctx.enter_context(tc.tile_pool(name="xp", bufs=2))
            xpbp = ctx.enter_context(tc.tile_pool(name="xpbp", bufs=3))
            xhp = ctx.enter_context(tc.tile_pool(name="xhp", bufs=1))
            ytp = ctx.enter_context(tc.tile_pool(name="ytp", bufs=2))
            qp = ctx.enter_context(tc.tile_pool(name="qp", bufs=2))
            kp = ctx.enter_context(tc.tile_pool(name="kp", bufs=2))
            vp = ctx.enter_context(tc.tile_pool(name="vp", bufs=2))
            ep = ctx.enter_context(tc.tile_pool(name="ep", bufs=1))
            astp = ctx.enter_context(tc.tile_pool(name="astp", bufs=1))
            a16p = ctx.enter_context(tc.tile_pool(name="a16p", bufs=1))
            x1p = ctx.enter_context(tc.tile_pool(name="x1p", bufs=1))
            h2tp = ctx.enter_context(tc.tile_pool(name="h2tp", bufs=1))
            gelp = ctx.enter_context(tc.tile_pool(name="gelp", bufs=1))
            op = ctx.enter_context(tc.tile_pool(name="op", bufs=2))
            sp = ctx.enter_context(tc.tile_pool(name="sp", bufs=2))
            rp = ctx.enter_context(tc.tile_pool(name="rp", bufs=1))
            tp = ctx.enter_context(tc.tile_pool(name="tp", bufs=2))
            dp = ctx.enter_context(tc.tile_pool(name="dp", bufs=2, space="DRAM"))
            ps_g = ctx.enter_context(tc.tile_pool(name="ps_g", bufs=3, space="PSUM"))
            ps_s = ctx.enter_context(tc.tile_pool(name="ps_s", bufs=3, space="PSUM"))
            ps_a = ctx.enter_context(tc.tile_pool(name="ps_a", bufs=2, space="PSUM"))

            # ---- constants ----
            qkvw_sb = consts.tile([128, 4, 3 * DIM], F16, name="qkvw_sb")
            nc.sync.dma_start(qkvw_sb[:], qkvw_d[:, :, :])
            projw_sb = consts.tile([128, 4, DIM], F16, name="projw_sb")
            nc.sync.dma_start(projw_sb[:], projw_d[:, :, :])
            w1_sb = consts.tile([128, 4, FF], F16, name="w1_sb")
            nc.sync.dma_start(w1_sb[:], w1_d[:, :, :])
            w2_sb = consts.tile([128, 16, DIM], F16, name="w2_sb")
            nc.sync.dma_start(w2_sb[:], w2_d[:, :, :])

            arow_col = consts.tile([128, 4], F32, name="arow_col")
            nc.sync.dma_start(arow_col[:], _col_view(arow_d.ap(), 0, 4))
            crow_col = consts.tile([128, 4], F32, name="crow_col")
            nc.sync.dma_start(crow_col[:], _col_view(crow_d.ap(), 0, 4))
            qkb_sb = consts.tile([128, 8], F32, name="qkb_sb")
            nc.sync.dma_start(qkb_sb[:], _col_view(qkb_d.ap(), 0, 8))
            b1_sb = consts.tile([128, 16], F32, name="b1_sb")
            nc.sync.dma_start(b1_sb[:], _col_view(b1_d.ap(), 0, 16))
            vb_bc = consts.tile([128, DIM], F32, name="vb_bc")
            nc.sync.dma_start(vb_bc[:], _bcast_row(vb_d.ap(), 0, DIM))
            projb_bc = consts.tile([128, DIM], F32, name="projb_bc")
            nc.sync.dma_start(projb_bc[:], _bcast_row(projb_d.ap(), 0, DIM))
            b2_bc = consts.tile([128, DIM], F32, name="b2_bc")
            nc.sync.dma_start(b2_bc[:], _bcast_row(b2_d.ap(), 0, DIM))
            eps_t = consts.tile([128, 1], F32, name="eps_t")
            nc.vector.memset(eps_t[:], EPS)
            ones_bf = consts.tile([1, 64], BF16, name="ones_bf")
            nc.vector.memset(ones_bf[:], 1.0)

            stages = {}   # g -> dict of tiles
            e_tiles = {}  # (h, j) -> E tile

            def qkv_ln(g):
                """x load + LN1 + transpose + modulate -> y16 (no matmuls).

                Emitted one group early so its ACT sqrt isn't queued behind
                the attention exps and PE always has QKV work ready."""
                st = {}
                xts, xpbs = [], []
                mv = tp.tile([128, 4, 2], F32, name=f"mv1_{g}", tag="mv1")
                for t in range(4):
                    xt = xp.tile([128, DIM], F32, name=f"x_{g}_{t}", tag=f"x{t}")
                    nc.sync.dma_start(xt[:], x_d[(g * 4 + t) * 128:(g * 4 + t + 1) * 128, :])
                    stats = tp.tile([128, 6], F32, name=f"st_{g}_{t}", tag=f"st{t}")
                    nc.vector.bn_stats(stats[:], xt[:])
                    nc.vector.bn_aggr(mv[:, t:t + 1, :], stats[:])
                    xts.append(xt)
                std = tp.tile([128, 4], F32, name=f"sd_{g}", tag="sd1")
                nc.scalar.activation(std[:], mv[:, :, 1:2], AF.Sqrt, bias=eps_t[:])
                rs = tp.tile([128, 4], F32, name=f"rs_{g}", tag="rs1")
                nc.vector.reciprocal(rs[:], std[:])
                xhT = ytp.tile([128, 4, GRP], F16, name=f"xhT_{g}", tag="xhT")
                for t in range(4):
                    xh = xhp.tile([128, DIM], F16, name=f"xh_{g}_{t}", tag=f"xh{t}")
                    nc.vector.tensor_scalar(xh[:], xts[t][:], mv[:, t:t + 1, 0:1], rs[:, t:t + 1],
                                            op0=AL.subtract, op1=AL.mult)
                    # x + projb precomputed (fp16) so x tiles die early
                    xpb = xpbp.tile([128, DIM], F16, name=f"xpb_{g}_{t}", tag=f"xpb{t}")
                    nc.vector.tensor_tensor(xpb[:], xts[t][:], projb_bc[:], op=AL.add)
                    xpbs.append(xpb)
                    # one batched transpose per token tile (sync ring)
                    nc.sync.dma_start_transpose(xhT[:, :, t * 128:(t + 1) * 128], xh[:])
                st["xpb"] = xpbs
                # modulate per chunk (arow/crow are per-partition on transposed side)
                y16 = ytp.tile([128, 4, GRP], F16, name=f"y16_{g}", tag="y16")
                for c in range(4):
                    nc.vector.tensor_scalar(y16[:, c, :], xhT[:, c, :],
                                            arow_col[:, c:c + 1], crow_col[:, c:c + 1],
                                            op0=AL.mult, op1=AL.add)
                st["y16"] = y16
                st["q"] = [None] * 4
                st["k"] = [None] * 4
                st["v"] = [None] * 4
                return st

            def qkv_piece_qk(g, m):
                """One QK output chunk for group g (4 MMs + bias)."""
                st = stages[g]
                P = ps_g.tile([128, GRP], F32, name=f"Pqk_{g}_{m}", tag="gemm")
                for c in range(4):
                    nc.tensor.matmul(P[:], qkvw_sb[:, c, m * 128:(m + 1) * 128],
                                     st["y16"][:, c, :], start=(c == 0), stop=(c == 3))
                pool = qp if m < 4 else kp
                nm = f"q_{g}_{m}" if m < 4 else f"k_{g}_{m-4}"
                tg = f"q{m}" if m < 4 else f"k{m-4}"
                sb = pool.tile([128, GRP], F16, name=nm, tag=tg)
                nc.vector.tensor_scalar_add(sb[:], P[:], qkb_sb[:, m:m + 1])
                if m < 4:
                    st["q"][m] = sb
                else:
                    st["k"][m - 4] = sb

            def qkv_piece_v(g, t):
                """One V token tile for group g (4 MMs + bias + ones col)."""
                st = stages[g]
                P = ps_g.tile([128, DIM], F32, name=f"Pv_{g}_{t}", tag="gemm")
                for c in range(4):
                    nc.tensor.matmul(P[:], st["y16"][:, c, t * 128:(t + 1) * 128],
                                     qkvw_sb[:, c, 2 * DIM:3 * DIM],
                                     start=(c == 0), stop=(c == 3))
                vt = vp.tile([128, HEADS, HD + 1], F16, name=f"v_{g}_{t}", tag=f"v{t}")
                nc.vector.memset(vt[:, :, HD:HD + 1], 1.0)
                nc.vector.tensor_tensor(
                    vt[:, :, 0:HD],
                    P[:].rearrange("p (h d) -> p h d", h=HEADS),
                    vb_bc[:].rearrange("p (h d) -> p h d", h=HEADS),
                    op=AL.add)
                st["v"][t] = vt

            def make_pieces(g):
                """QKV matmul closures for group g, ordered so the chunks the
                next attention phase needs first are produced first."""
                ps = []
                for hp in range(4):
                    ps.append(lambda m=hp: qkv_piece_qk(g, m))
                    ps.append(lambda m=4 + hp: qkv_piece_qk(g, m))
                    ps.append(lambda t=hp: qkv_piece_v(g, t))
                return ps

            def attn_stage(gp, pieces):
                # key blocks computed this phase
                j_list = [j for j in range(4 * gp + 1, 4 * gp + 5) if j < nW]
                if gp == 0:
                    j_list = [0] + j_list
                attn_f16 = {}
                a16 = a16p.tile([128, 4, GRP], F16, name=f"a16_{gp}", tag="a16")
                pieces = list(pieces)
                for hp in range(4):
                    heads = (2 * hp, 2 * hp + 1)
                    af = astp.tile([128, GRP], F16, name=f"af_{gp}_{hp}", tag=f"af{hp}")
                    attn_f16[hp] = af
                    P_av = {}
                    for h in heads:
                        P_av[h] = ps_a.tile([65, 4, 128], F32, name=f"Pav_{gp}_{h}", tag="av")
                    for j in j_list:
                        gj, sj = divmod(j, 4)
                        qlo = max(0, j - 1)
                        qhi = min(nW - 1, j + 1)
                        ncols = (qhi - qlo + 1) * WIN
                        for h in heads:
                            off = (h % 2) * 64
                            P_sim = ps_s.tile([128, 384], F32, name=f"Ps_{gp}_{h}_{j}", tag="sim")
                            # q columns may span two group tiles -> split segments
                            w0 = qlo
                            while w0 <= qhi:
                                gq = w0 // 4
                                wend = min(qhi, gq * 4 + 3)
                                c0 = (w0 % 4) * WIN
                                c1 = (wend % 4 + 1) * WIN
                                dst0 = (w0 - qlo) * WIN
                                nc.tensor.matmul(
                                    P_sim[:, dst0:dst0 + (c1 - c0)],
                                    stages[gj]["k"][hp][off:off + 64, sj * 128:(sj + 1) * 128],
                                    stages[gq]["q"][hp][off:off + 64, c0:c1],
                                    start=True, stop=True)
                                w0 = wend + 1
                            E = ep.tile([128, 384], F16, name=f"E_{gp}_{h}_{j}",
                                        tag=f"E{h}_{j % 3}")
                            nc.scalar.activation(E[:, 0:ncols], P_sim[:, 0:ncols],
                                                 AF.Exp, scale=SIMSCALE)
                            e_tiles[(h, j)] = E
                        # AV for completed windows
                        av_ws = []
                        if 4 * gp <= j - 1 <= 4 * gp + 3:
                            av_ws.append(j - 1)
                        if j == j_list[-1] and j == nW - 1:
                            av_ws.append(nW - 1)
                        for w in av_ws:
                            jjs = [jj for jj in (w - 1, w, w + 1) if 0 <= jj < nW]
                            for h in heads:
                                for ji, jj in enumerate(jjs):
                                    gjj, sjj = divmod(jj, 4)
                                    colofs = (w - max(0, jj - 1)) * WIN
                                    nc.tensor.matmul(
                                        P_av[h][:, w % 4, :],
                                        stages[gjj]["v"][sjj][:, h, :],
                                        e_tiles[(h, jj)][:, colofs:colofs + WIN],
                                        start=(ji == 0), stop=(ji == len(jjs) - 1))
                        # interleave one QKV piece of the next group between
                        # attention steps so the PE queue never blocks long
                        if pieces:
                            pieces.pop(0)()
                    # evacuate PSUM: unnormalized attn (DVE when aligned) + sums
                    P_rbc = ps_g.tile([128, GRP], F32, name=f"Prbc_{gp}_{hp}", tag="gemm")
                    for h in heads:
                        off = (h % 2) * 64
                        src = P_av[h][0:64, :, :].rearrange("p a b -> p (a b)")
                        if h % 2 == 0:
                            nc.vector.tensor_copy(af[0:64, :], src)
                        else:
                            nc.scalar.activation(af[64:128, :], src, AF.Identity)
                        ss = sp.tile([1, GRP], F32, name=f"ss_{gp}_{h}", tag="ss")
                        nc.scalar.activation(ss[:],
                                             P_av[h][64:65, :, :].rearrange("p a b -> p (a b)"),
                                             AF.Identity)
                        # on-chip softmax denominators: fast reciprocal, cast
                        # bf16, PE ones-outer-product broadcast across the 64
                        # feature partitions of this head
                        rr = sp.tile([1, GRP], F32, name=f"rr_{gp}_{h}", tag="rr")
                        nc.vector.reciprocal_approx_fast(rr[:], ss[:])
                        rrb = sp.tile([1, GRP], BF16, name=f"rrb_{gp}_{h}", tag="rrb")
                        with nc.allow_low_precision(reason="softmax recip bcast bf16"):
                            nc.vector.tensor_copy(rrb[:], rr[:])
                        nc.tensor.matmul(P_rbc[off:off + 64, :], ones_bf[:], rrb[:],
                                         start=True, stop=True)
                    nc.vector.tensor_tensor(a16[:, hp, :], attn_f16[hp][:], P_rbc[:],
                                            op=AL.mult)
                for pc in pieces:
                    pc()
                return a16

            def proj_mlp_stage(gp, a16):
                cur = stages[gp]
                # proj + residual -> x1 (token-major f32)
                x1_t = []
                for t in range(4):
                    P = ps_g.tile([128, DIM], F32, name=f"Ppr_{gp}_{t}", tag="gemm")
                    for c in range(4):
                        nc.tensor.matmul(P[:], a16[:, c, t * 128:(t + 1) * 128],
                                         projw_sb[:, c, :], start=(c == 0), stop=(c == 3))
                    x1 = x1p.tile([128, DIM], F32, name=f"x1_{gp}_{t}", tag=f"x1{t}")
                    nc.vector.tensor_tensor(x1[:], P[:], cur["xpb"][t][:], op=AL.add)
                    x1_t.append(x1)
                # LN2 -> x_hat2 fp16, batched transpose (sync ring)
                mv2 = tp.tile([128, 4, 2], F32, name=f"mv2_{gp}", tag="mv2")
                for t in range(4):
                    stats = tp.tile([128, 6], F32, name=f"st2_{gp}_{t}", tag=f"st2{t}")
                    nc.vector.bn_stats(stats[:], x1_t[t][:])
                    nc.vector.bn_aggr(mv2[:, t:t + 1, :], stats[:])
                std2 = tp.tile([128, 4], F32, name=f"sd2_{gp}", tag="sd2")
                nc.scalar.activation(std2[:], mv2[:, :, 1:2], AF.Sqrt, bias=eps_t[:])
                rs2 = tp.tile([128, 4], F32, name=f"rs2_{gp}", tag="rs2")
                nc.vector.reciprocal(rs2[:], std2[:])
                h2T = h2tp.tile([128, 4, GRP], F16, name=f"h2T_{gp}", tag="h2T")
                for t in range(4):
                    xh2 = xhp.tile([128, DIM], F16, name=f"xh2_{gp}_{t}", tag=f"xh2{t}")
                    nc.vector.tensor_scalar(xh2[:], x1_t[t][:], mv2[:, t:t + 1, 0:1], rs2[:, t:t + 1],
                                            op0=AL.subtract, op1=AL.mult)
                    nc.sync.dma_start_transpose(h2T[:, :, t * 128:(t + 1) * 128], xh2[:])
                # MLP1 + gelu (feature-major) fp16
                gel = gelp.tile([128, 16, GRP], F16, name=f"gel_{gp}", tag="gel")
                for f in range(16):
                    P = ps_g.tile([128, GRP], F32, name=f"Pm1_{gp}_{f}", tag="gemm")
                    for c in range(4):
                        nc.tensor.matmul(P[:], w1_sb[:, c, f * 128:(f + 1) * 128],
                                         h2T[:, c, :], start=(c == 0), stop=(c == 3))
                    nc.scalar.activation(gel[:, f, :], P[:], AF.Gelu, bias=b1_sb[:, f:f + 1])
                # MLP2 + bias + residual -> out (token-major)
                for t in range(4):
                    P = ps_g.tile([128, DIM], F32, name=f"Pm2_{gp}_{t}", tag="gemm")
                    for f in range(16):
                        nc.tensor.matmul(P[:], gel[:, f, t * 128:(t + 1) * 128],
                                         w2_sb[:, f, :], start=(f == 0), stop=(f == 15))
                    x1b = tp.tile([128, DIM], F32, name=f"x1b_{gp}_{t}", tag="x1b")
                    nc.vector.tensor_tensor(x1b[:], x1_t[t][:], b2_bc[:], op=AL.add)
                    ot = op.tile([128, DIM], F32, name=f"o_{gp}_{t}", tag="o")
                    nc.vector.tensor_tensor(ot[:], P[:], x1b[:], op=AL.add)
                    nc.sync.dma_start(out_d[(gp * 4 + t) * 128:(gp * 4 + t + 1) * 128, :], ot[:])

            stages[0] = qkv_ln(0)
            if n_groups > 1:
                stages[1] = qkv_ln(1)
            for pc in make_pieces(0):
                pc()
            for gp in range(n_groups):
                pieces = make_pieces(gp + 1) if gp + 1 < n_groups else []
                a16 = attn_stage(gp, pieces)
                if gp + 2 < n_groups:
                    stages[gp + 2] = qkv_ln(gp + 2)
                proj_mlp_stage(gp, a16)

    nc.compile()
    return nc


_cache = {}


def _get_nc(n_tok):
    if n_tok not in _cache:
        _cache[n_tok] = build(n_tok)
    return _cache[n_tok]


def _prep_in_maps(inputs):
    return _prep(**inputs)


def _w16(w, chunks):
    """[K, M] f32 -> [128, K//128, M] fp16."""
    K, M = w.shape
    assert K == 128 * chunks
    return np.ascontiguousarray(
        w.astype(np.float16).reshape(chunks, 128, M).transpose(1, 0, 2))


def _prep(x, t_emb, ln1_g, ln1_b, qkv_w, qkv_b, proj_w, proj_b,
          ln2_g, ln2_b, mlp_w1, mlp_b1, mlp_w2, mlp_b2, time_w, time_b):
    x = np.asarray(x, dtype=np.float32)
    t_emb = np.asarray(t_emb, np.float32)
    # host: modulation rows (tiny), fold ln1 gamma/beta
    s = t_emb / (1.0 + np.exp(-t_emb))           # silu
    ss = s @ np.asarray(time_w, np.float32) + np.asarray(time_b, np.float32)
    scale, shift = ss[:, :DIM], ss[:, DIM:]
    g1 = np.asarray(ln1_g, np.float32)
    be1 = np.asarray(ln1_b, np.float32)
    arow = g1[None, :] * (1.0 + scale)                      # [B, 512]
    crow = be1[None, :] * (1.0 + scale) + shift             # [B, 512]
    # fold ln2 gamma/beta into mlp_w1/b1
    g2 = np.asarray(ln2_g, np.float32)
    be2 = np.asarray(ln2_b, np.float32)
    w1f = np.asarray(mlp_w1, np.float32) * g2[:, None]
    b1f = be2 @ np.asarray(mlp_w1, np.float32) + np.asarray(mlp_b1, np.float32)

    qkvw16 = _w16(np.asarray(qkv_w, np.float32), 4)
    projw16 = _w16(np.asarray(proj_w, np.float32), 4)
    w116 = _w16(w1f, 4)
    w216 = _w16(np.asarray(mlp_w2, np.float32), 16)
    qkvb = np.asarray(qkv_b, np.float32)
    qkb = np.ascontiguousarray(qkvb[:2 * DIM])
    vb = np.ascontiguousarray(qkvb[2 * DIM:])
    projb = np.asarray(proj_b, np.float32)
    b2 = np.asarray(mlp_b2, np.float32)

    in_maps = []
    nb = x.shape[0]
    for b in range(nb):
        in_maps.append({
            "x": np.ascontiguousarray(x[b]),
            "arow": np.ascontiguousarray(arow[b]),
            "crow": np.ascontiguousarray(crow[b]),
            "qkvw": qkvw16, "qkb": qkb, "vb": vb,
            "projw": projw16, "projb": projb,
            "w1": w116, "b1": b1f, "w2": w216, "b2": b2,
        })
    return in_maps


def kernel(**inputs):
    in_maps = _prep_in_maps(inputs)
    n_tok = in_maps[0]["x"].shape[0]
    nc = _get_nc(n_tok)
    nb = len(in_maps)
    res = bass_utils.run_bass_kernel_spmd(nc, in_maps, core_ids=list(range(nb)))
    out = np.stack([res.results[b]["out"] for b in range(nb)], axis=0)
    return out


# revision 48
# speedup vs baseline: 1.3085x; 1.0092x over previous
"""Trainium2 Bass kernel for a local-attention transformer block (v3, fp16).

Computes, per batch element (one NeuronCore each, 8 cores):
  ss = silu(t_emb) @ time_w + time_b ;  scale, shift = split(ss)
  y  = LN(x) * (1+scale) + shift                       (ln1 g/b host-folded)
  q,k,v = y @ qkv_w + qkv_b  (heads=8, d=64)
  attn: each 128-token window attends to [prev|cur|next] windows
  x1 = x + attn @ proj_w + proj_b
  out = x1 + gelu(LN2(x1) @ w1 + b1') @ w2 + b2        (ln2 g/b folded into w1/b1)

v3 strategy (evolved from the 2.18 ms fp16 baseline):
  - All GEMMs fp16 (measured: fp16=bf16=fp8 all stream 216 ns per N=512 matmul;
    DoubleRow's 256-col LDWEIGHTS doesn't background-load, so fp8 gains nothing).
    Weights stored [128, n_chunks, out] fp16, activations transposed to
    [128, n_chunks, 512] fp16 chunk tiles.
  - Attention key-block-major: per (head, key block j) ONE sim matmul of
    N<=384 (q windows j-1..j+1, keys on partitions), exp into an E tile
    reused by 3 AV windows; AV accumulates [65, 4win, 128] PSUM per head
    (ones column folded into v_aug produces softmax denominators).
  - LN transposes x_hat fp16 via ONE batched DMA transpose per token tile
    ([128,512] -> [128,4,128], same 1.2us as a 128x128 transpose); modulate
    fused into a per-chunk tensor_scalar on the transposed side.
  - ACT engine runs ONLY Exp/Gelu/Sqrt (no Identity copies, no DMA issue):
    PSUM evacuations + bias adds on DVE (cross-partition DVE copies verified),
    per-head softmax reciprocal on DVE directly from PSUM row 64.
  - All DMA on the sync ring (transposes batched); scalar ring unused so the
    ACT queue stays clean.
"""

import numpy as np
from contextlib import ExitStack

import concourse.bass as bass
import concourse.tile as tile
from concourse import bacc, mybir
from concourse import bass_utils

F32 = mybir.dt.float32
F16 = mybir.dt.float16
BF16 = mybir.dt.bfloat16
AF = mybir.ActivationFunctionType
AL = mybir.AluOpType

DIM = 512
HEADS = 8
HD = 64
FF = 2048
WIN = 128
B = 8
NTOK = 8192
EPS = 1e-5
GRP = 512  # tokens per group (4 windows)
SIMSCALE = float(HD) ** -0.5


def _col_view(dram_ap, offset, ncol):
    """AP reading dram vector [128*ncol] as [128, ncol] feature-major columns."""
    return bass.AP(tensor=dram_ap.tensor, offset=offset, ap=[[1, 128], [128, ncol]])


def _bcast_row(dram_ap, offset, n):
    """AP reading dram vector [n] broadcast across 128 partitions."""
    return bass.AP(tensor=dram_ap.tensor, offset=offset, ap=[[0, 128], [1, n]])


def build(n_tok=NTOK):
    n_groups = n_tok // GRP
    nW = n_tok // WIN
    nc = bacc.Bacc("TRN2", target_bir_lowering=False, debug=False)

    x_d = nc.dram_tensor("x", [n_tok, DIM], F32, kind="ExternalInput")
    arow_d = nc.dram_tensor("arow", [DIM], F32, kind="ExternalInput")
    crow_d = nc.dram_tensor("crow", [DIM], F32, kind="ExternalInput")
    qkvw_d = nc.dram_tensor("qkvw", [128, 4, 3 * DIM], F16, kind="ExternalInput")
    qkb_d = nc.dram_tensor("qkb", [2 * DIM], F32, kind="ExternalInput")
    vb_d = nc.dram_tensor("vb", [DIM], F32, kind="ExternalInput")
    projw_d = nc.dram_tensor("projw", [128, 4, DIM], F16, kind="ExternalInput")
    projb_d = nc.dram_tensor("projb", [DIM], F32, kind="ExternalInput")
    w1_d = nc.dram_tensor("w1", [128, 4, FF], F16, kind="ExternalInput")
    b1_d = nc.dram_tensor("b1", [FF], F32, kind="ExternalInput")
    w2_d = nc.dram_tensor("w2", [128, 16, DIM], F16, kind="ExternalInput")
    b2_d = nc.dram_tensor("b2", [DIM], F32, kind="ExternalInput")
    out_d = nc.dram_tensor("out", [n_tok, DIM], F32, kind="ExternalOutput")

    with tile.TileContext(nc) as tc:
        with ExitStack() as ctx:
            consts = ctx.enter_context(tc.tile_pool(name="consts", bufs=1))
            xp = ctx.enter_context(tc.tile_pool(name="xp", bufs=3))
            xpbp = ctx.enter_context(tc.tile_pool(name="xpbp", bufs=3))
            xhp = ctx.enter_context(tc.tile_pool(name="xhp", bufs=1))
            ytp = ctx.enter_context(tc.tile_pool(name="ytp", bufs=2))
            xhtp = ctx.enter_context(tc.tile_pool(name="xhtp", bufs=1))
            qp = ctx.enter_context(tc.tile_pool(name="qp", bufs=2))
            kp = ctx.enter_context(tc.tile_pool(name="kp", bufs=2))
            vp = ctx.enter_context(tc.tile_pool(name="vp", bufs=2))
            ep = ctx.enter_context(tc.tile_pool(name="ep", bufs=1))
            astp = ctx.enter_context(tc.tile_pool(name="astp", bufs=1))
            a16p = ctx.enter_context(tc.tile_pool(name="a16p", bufs=1))
            x1p = ctx.enter_context(tc.tile_pool(name="x1p", bufs=1))
            h2tp = ctx.enter_context(tc.tile_pool(name="h2tp", bufs=1))
            gelp = ctx.enter_context(tc.tile_pool(name="gelp", bufs=1))
            op = ctx.enter_context(tc.tile_pool(name="op", bufs=2))
            sp = ctx.enter_context(tc.tile_pool(name="sp", bufs=2))
            tp = ctx.enter_context(tc.tile_pool(name="tp", bufs=2))
            dp = ctx.enter_context(tc.tile_pool(name="dp", bufs=2, space="DRAM"))
            ps_g = ctx.enter_context(tc.tile_pool(name="ps_g", bufs=3, space="PSUM"))
            ps_s = ctx.enter_context(tc.tile_pool(name="ps_s", bufs=3, space="PSUM"))
            ps_a = ctx.enter_context(tc.tile_pool(name="ps_a", bufs=2, space="PSUM"))

            # ---- constants ----
            qkvw_sb = consts.tile([128, 4, 3 * DIM], F16, name="qkvw_sb")
            nc.sync.dma_start(qkvw_sb[:], qkvw_d[:, :, :])
            projw_sb = consts.tile([128, 4, DIM], F16, name="projw_sb")
            nc.sync.dma_start(projw_sb[:], projw_d[:, :, :])
            w1_sb = consts.tile([128, 4, FF], F16, name="w1_sb")
            nc.sync.dma_start(w1_sb[:], w1_d[:, :, :])
            w2_sb = consts.tile([128, 16, DIM], F16, name="w2_sb")
            nc.sync.dma_start(w2_sb[:], w2_d[:, :, :])

            arow_col = consts.tile([128, 4], F32, name="arow_col")
            nc.sync.dma_start(arow_col[:], _col_view(arow_d.ap(), 0, 4))
            crow_col = consts.tile([128, 4], F32, name="crow_col")
            nc.sync.dma_start(crow_col[:], _col_view(crow_d.ap(), 0, 4))
            qkb_sb = consts.tile([128, 8], F32, name="qkb_sb")
            nc.sync.dma_start(qkb_sb[:], _col_view(qkb_d.ap(), 0, 8))
            b1_sb = consts.tile([128, 16], F32, name="b1_sb")
            nc.sync.dma_start(b1_sb[:], _col_view(b1_d.ap(), 0, 16))
            vb_bc = consts.tile([128, DIM], F32, name="vb_bc")
            nc.sync.dma_start(vb_bc[:], _bcast_row(vb_d.ap(), 0, DIM))
            projb_bc = consts.tile([128, DIM], F32, name="projb_bc")
            nc.sync.dma_start(projb_bc[:], _bcast_row(projb_d.ap(), 0, DIM))
            b2_bc = consts.tile([128, DIM], F32, name="b2_bc")
            nc.sync.dma_start(b2_bc[:], _bcast_row(b2_d.ap(), 0, DIM))
            eps_t = consts.tile([128, 1], F32, name="eps_t")
            nc.vector.memset(eps_t[:], EPS)
            ones_bf = consts.tile([1, 64], BF16, name="ones_bf")
            nc.vector.memset(ones_bf[:], 1.0)

            stages = {}   # g -> dict of tiles
            e_tiles = {}  # (h, j) -> E tile

            def qkv_ln(g):
                """x load + LN1 + transpose + modulate -> y16 (no matmuls).

                Emitted one group early so its ACT sqrt isn't queued behind
                the attention exps and PE always has QKV work ready."""
                st = {}
                xts, xpbs = [], []
                mv = tp.tile([128, 4, 2], F32, name=f"mv1_{g}", tag="mv1")
                for t in range(4):
                    xt = xp.tile([128, DIM], F32, name=f"x_{g}_{t}", tag=f"x{t}")
                    nc.sync.dma_start(xt[:], x_d[(g * 4 + t) * 128:(g * 4 + t + 1) * 128, :])
                    stats = tp.tile([128, 6], F32, name=f"st_{g}_{t}", tag=f"st{t}")
                    nc.vector.bn_stats(stats[:], xt[:])
                    nc.vector.bn_aggr(mv[:, t:t + 1, :], stats[:])
                    xts.append(xt)
                std = tp.tile([128, 4], F32, name=f"sd_{g}", tag="sd1")
                nc.scalar.activation(std[:], mv[:, :, 1:2], AF.Sqrt, bias=eps_t[:])
                rs = tp.tile([128, 4], F32, name=f"rs_{g}", tag="rs1")
                nc.vector.reciprocal(rs[:], std[:])
                xhT = xhtp.tile([128, 4, GRP], F16, name=f"xhT_{g}", tag="xhT")
                for t in range(4):
                    xh = xhp.tile([128, DIM], F16, name=f"xh_{g}_{t}", tag=f"xh{t}")
                    nc.vector.tensor_scalar(xh[:], xts[t][:], mv[:, t:t + 1, 0:1], rs[:, t:t + 1],
                                            op0=AL.subtract, op1=AL.mult)
                    # x + projb precomputed (fp16) so x tiles die early
                    xpb = xpbp.tile([128, DIM], F16, name=f"xpb_{g}_{t}", tag=f"xpb{t}")
                    nc.vector.tensor_tensor(xpb[:], xts[t][:], projb_bc[:], op=AL.add)
                    xpbs.append(xpb)
                    # one batched transpose per token tile (sync ring)
                    nc.sync.dma_start_transpose(xhT[:, :, t * 128:(t + 1) * 128], xh[:])
                st["xpb"] = xpbs
                # modulate per chunk (arow/crow are per-partition on transposed side)
                y16 = ytp.tile([128, 4, GRP], F16, name=f"y16_{g}", tag="y16")
                for c in range(4):
                    nc.vector.tensor_scalar(y16[:, c, :], xhT[:, c, :],
                                            arow_col[:, c:c + 1], crow_col[:, c:c + 1],
                                            op0=AL.mult, op1=AL.add)
                st["y16"] = y16
                st["q"] = [None] * 4
                st["k"] = [None] * 4
                st["v"] = [None] * 4
                return st

            def qkv_piece_qk(g, m):
                """One QK output chunk for group g (4 MMs + bias)."""
                st = stages[g]
                P = ps_g.tile([128, GRP], F32, name=f"Pqk_{g}_{m}", tag="gemm")
                for c in range(4):
                    nc.tensor.matmul(P[:], qkvw_sb[:, c, m * 128:(m + 1) * 128],
                                     st["y16"][:, c, :], start=(c == 0), stop=(c == 3))
                pool = qp if m < 4 else kp
                nm = f"q_{g}_{m}" if m < 4 else f"k_{g}_{m-4}"
                tg = f"q{m}" if m < 4 else f"k{m-4}"
                sb = pool.tile([128, GRP], F16, name=nm, tag=tg)
                nc.vector.tensor_scalar_add(sb[:], P[:], qkb_sb[:, m:m + 1])
                if m < 4:
                    st["q"][m] = sb
                else:
                    st["k"][m - 4] = sb

            def qkv_piece_v(g, t):
                """One V token tile for group g (4 MMs + bias + ones col)."""
                st = stages[g]
                P = ps_g.tile([128, DIM], F32, name=f"Pv_{g}_{t}", tag="gemm")
                for c in range(4):
                    nc.tensor.matmul(P[:], st["y16"][:, c, t * 128:(t + 1) * 128],
                                     qkvw_sb[:, c, 2 * DIM:3 * DIM],
                                     start=(c == 0), stop=(c == 3))
                vt = vp.tile([128, HEADS, HD + 1], F16, name=f"v_{g}_{t}", tag=f"v{t}")
                nc.vector.memset(vt[:, :, HD:HD + 1], 1.0)
                nc.vector.tensor_tensor(
                    vt[:, :, 0:HD],
                    P[:].rearrange("p (h d) -> p h d", h=HEADS),
                    vb_bc[:].rearrange("p (h d) -> p h d", h=HEADS),
                    op=AL.add)
                st["v"][t] = vt

            def make_pieces(g):
                """QKV matmul closures for group g, ordered so the chunks the
                next attention phase needs first are produced first."""
                ps = []
                for hp in range(4):
                    ps.append(lambda m=hp: qkv_piece_qk(g, m))
                    ps.append(lambda m=4 + hp: qkv_piece_qk(g, m))
                    ps.append(lambda t=hp: qkv_piece_v(g, t))
                return ps

            def attn_stage(gp, pieces):
                # key blocks computed this phase
                j_list = [j for j in range(4 * gp + 1, 4 * gp + 5) if j < nW]
                if gp == 0:
                    j_list = [0] + j_list
                attn_f16 = {}
                a16 = a16p.tile([128, 4, GRP], F16, name=f"a16_{gp}", tag="a16")
                pieces = list(pieces)
                for hp in range(4):
                    heads = (2 * hp, 2 * hp + 1)
                    af = astp.tile([128, GRP], F16, name=f"af_{gp}_{hp}", tag=f"af{hp}")
                    attn_f16[hp] = af
                    P_av = {}
                    for h in heads:
                        P_av[h] = ps_a.tile([65, 4, 128], F32, name=f"Pav_{gp}_{h}", tag="av")
                    for j in j_list:
                        gj, sj = divmod(j, 4)
                        qlo = max(0, j - 1)
                        qhi = min(nW - 1, j + 1)
                        ncols = (qhi - qlo + 1) * WIN
                        for h in heads:
                            off = (h % 2) * 64
                            P_sim = ps_s.tile([128, 384], F32, name=f"Ps_{gp}_{h}_{j}", tag="sim")
                            # q columns may span two group tiles -> split segments
                            w0 = qlo
                            while w0 <= qhi:
                                gq = w0 // 4
                                wend = min(qhi, gq * 4 + 3)
                                c0 = (w0 % 4) * WIN
                                c1 = (wend % 4 + 1) * WIN
                                dst0 = (w0 - qlo) * WIN
                                nc.tensor.matmul(
                                    P_sim[:, dst0:dst0 + (c1 - c0)],
                                    stages[gj]["k"][hp][off:off + 64, sj * 128:(sj + 1) * 128],
                                    stages[gq]["q"][hp][off:off + 64, c0:c1],
                                    start=True, stop=True)
                                w0 = wend + 1
                            E = ep.tile([128, 384], F16, name=f"E_{gp}_{h}_{j}",
                                        tag=f"E{h}_{j % 3}")
                            nc.scalar.activation(E[:, 0:ncols], P_sim[:, 0:ncols],
                                                 AF.Exp, scale=SIMSCALE)
                            e_tiles[(h, j)] = E
                        # AV for completed windows
                        av_ws = []
                        if 4 * gp <= j - 1 <= 4 * gp + 3:
                            av_ws.append(j - 1)
                        if j == j_list[-1] and j == nW - 1:
                            av_ws.append(nW - 1)
                        for w in av_ws:
                            jjs = [jj for jj in (w - 1, w, w + 1) if 0 <= jj < nW]
                            for h in heads:
                                for ji, jj in enumerate(jjs):
                                    gjj, sjj = divmod(jj, 4)
                                    colofs = (w - max(0, jj - 1)) * WIN
                                    nc.tensor.matmul(
                                        P_av[h][:, w % 4, :],
                                        stages[gjj]["v"][sjj][:, h, :],
                                        e_tiles[(h, jj)][:, colofs:colofs + WIN],
                                        start=(ji == 0), stop=(ji == len(jjs) - 1))
                        # interleave one QKV piece of the next group between
                        # attention steps so the PE queue never blocks long
                        if pieces:
                            pieces.pop(0)()
                    # evacuate PSUM: unnormalized attn (DVE when aligned) + sums
                    P_rbc = ps_g.tile([128, GRP], F32, name=f"Prbc_{gp}_{hp}", tag="gemm")
                    for h in heads:
                        off = (h % 2) * 64
                        src = P_av[h][0:64, :, :].rearrange("p a b -> p (a b)")
                        if h % 2 == 0:
                            nc.vector.tensor_copy(af[0:64, :], src)
                        else:
                            nc.scalar.activation(af[64:128, :], src, AF.Identity)
                        ss = sp.tile([1, GRP], F32, name=f"ss_{gp}_{h}", tag="ss")
                        nc.scalar.activation(ss[:],
                                             P_av[h][64:65, :, :].rearrange("p a b -> p (a b)"),
                                             AF.Identity)
                        # on-chip softmax denominators: fast reciprocal, cast
                        # bf16, PE ones-outer-product broadcast across the 64
                        # feature partitions of this head
                        rr = sp.tile([1, GRP], F32, name=f"rr_{gp}_{h}", tag="rr")
                        nc.vector.reciprocal_approx_fast(rr[:], ss[:])
                        rrb = sp.tile([1, GRP], BF16, name=f"rrb_{gp}_{h}", tag="rrb")
                        with nc.allow_low_precision(reason="softmax recip bcast bf16"):
                            nc.vector.tensor_copy(rrb[:], rr[:])
                        nc.tensor.matmul(P_rbc[off:off + 64, :], ones_bf[:], rrb[:],
                                         start=True, stop=True)
                    nc.vector.tensor_tensor(a16[:, hp, :], attn_f16[hp][:], P_rbc[:],
                                            op=AL.mult)
                for pc in pieces:
                    pc()
                return a16

            def proj_ln2(gp, a16):
                cur = stages[gp]
                # proj + residual -> x1 (token-major f32)
                x1_t = []
                for t in range(4):
                    P = ps_g.tile([128, DIM], F32, name=f"Ppr_{gp}_{t}", tag="gemm")
                    for c in range(4):
                        nc.tensor.matmul(P[:], a16[:, c, t * 128:(t + 1) * 128],
                                         projw_sb[:, c, :], start=(c == 0), stop=(c == 3))
                    x1 = x1p.tile([128, DIM], F32, name=f"x1_{gp}_{t}", tag=f"x1{t}")
                    nc.vector.tensor_tensor(x1[:], P[:], cur["xpb"][t][:], op=AL.add)
                    x1_t.append(x1)
                # LN2 -> x_hat2 fp16, batched transposes split across rings
                mv2 = tp.tile([128, 4, 2], F32, name=f"mv2_{gp}", tag="mv2")
                for t in range(4):
                    stats = tp.tile([128, 6], F32, name=f"st2_{gp}_{t}", tag=f"st2{t}")
                    nc.vector.bn_stats(stats[:], x1_t[t][:])
                    nc.vector.bn_aggr(mv2[:, t:t + 1, :], stats[:])
                std2 = tp.tile([128, 4], F32, name=f"sd2_{gp}", tag="sd2")
                nc.scalar.activation(std2[:], mv2[:, :, 1:2], AF.Sqrt, bias=eps_t[:])
                rs2 = tp.tile([128, 4], F32, name=f"rs2_{gp}", tag="rs2")
                nc.vector.reciprocal(rs2[:], std2[:])
                h2T = h2tp.tile([128, 4, GRP], F16, name=f"h2T_{gp}", tag="h2T")
                for t in range(4):
                    xh2 = xhp.tile([128, DIM], F16, name=f"xh2_{gp}_{t}", tag=f"xh2{t}")
                    nc.vector.tensor_scalar(xh2[:], x1_t[t][:], mv2[:, t:t + 1, 0:1], rs2[:, t:t + 1],
                                            op0=AL.subtract, op1=AL.mult)
                    nc.sync.dma_start_transpose(h2T[:, :, t * 128:(t + 1) * 128], xh2[:])
                return x1_t, h2T

            def mlp_stage(gp, x1_t, h2T):
                # MLP1 + gelu (feature-major) fp16
                gel = gelp.tile([128, 16, GRP], F16, name=f"gel_{gp}", tag="gel")
                for f in range(16):
                    P = ps_g.tile([128, GRP], F32, name=f"Pm1_{gp}_{f}", tag="gemm")
                    for c in range(4):
                        nc.tensor.matmul(P[:], w1_sb[:, c, f * 128:(f + 1) * 128],
                                         h2T[:, c, :], start=(c == 0), stop=(c == 3))
                    nc.scalar.activation(gel[:, f, :], P[:], AF.Gelu, bias=b1_sb[:, f:f + 1])
                # MLP2 + bias + residual -> out (token-major)
                for t in range(4):
                    P = ps_g.tile([128, DIM], F32, name=f"Pm2_{gp}_{t}", tag="gemm")
                    for f in range(16):
                        nc.tensor.matmul(P[:], gel[:, f, t * 128:(t + 1) * 128],
                                         w2_sb[:, f, :], start=(f == 0), stop=(f == 15))
                    x1b = tp.tile([128, DIM], F32, name=f"x1b_{gp}_{t}", tag="x1b")
                    nc.vector.tensor_tensor(x1b[:], x1_t[t][:], b2_bc[:], op=AL.add)
                    ot = op.tile([128, DIM], F32, name=f"o_{gp}_{t}", tag="o")
                    nc.vector.tensor_tensor(ot[:], P[:], x1b[:], op=AL.add)
                    nc.sync.dma_start(out_d[(gp * 4 + t) * 128:(gp * 4 + t + 1) * 128, :], ot[:])

            stages[0] = qkv_ln(0)
            if n_groups > 1:
                stages[1] = qkv_ln(1)
            for pc in make_pieces(0):
                pc()
            for gp in range(n_groups):
                pieces = make_pieces(gp + 1) if gp + 1 < n_groups else []
                a16 = attn_stage(gp, pieces)
                x1_t, h2T = proj_ln2(gp, a16)
                if gp + 2 < n_groups:
                    stages[gp + 2] = qkv_ln(gp + 2)
                mlp_stage(gp, x1_t, h2T)

    nc.compile()
    return nc


_cache = {}


def _get_nc(n_tok):
    if n_tok not in _cache:
        _cache[n_tok] = build(n_tok)
    return _cache[n_tok]


def _prep_in_maps(inputs):
    return _prep(**inputs)


def _w16(w, chunks):
    """[K, M] f32 -> [128, K//128, M] fp16."""
    K, M = w.shape
    assert K == 128 * chunks
    return np.ascontiguousarray(
        w.astype(np.float16).reshape(chunks, 128, M).transpose(1, 0, 2))


def _prep(x, t_emb, ln1_g, ln1_b, qkv_w, qkv_b, proj_w, proj_b,
          ln2_g, ln2_b, mlp_w1, mlp_b1, mlp_w2, mlp_b2, time_w, time_b):
    x = np.asarray(x, dtype=np.float32)
    t_emb = np.asarray(t_emb, np.float32)
    # host: modulation rows (tiny), fold ln1 gamma/beta
    s = t_emb / (1.0 + np.exp(-t_emb))           # silu
    ss = s @ np.asarray(time_w, np.float32) + np.asarray(time_b, np.float32)
    scale, shift = ss[:, :DIM], ss[:, DIM:]
    g1 = np.asarray(ln1_g, np.float32)
    be1 = np.asarray(ln1_b, np.float32)
    arow = g1[None, :] * (1.0 + scale)                      # [B, 512]
    crow = be1[None, :] * (1.0 + scale) + shift             # [B, 512]
    # fold ln2 gamma/beta into mlp_w1/b1
    g2 = np.asarray(ln2_g, np.float32)
    be2 = np.asarray(ln2_b, np.float32)
    w1f = np.asarray(mlp_w1, np.float32) * g2[:, None]
    b1f = be2 @ np.asarray(mlp_w1, np.float32) + np.asarray(mlp_b1, np.float32)

    qkvw16 = _w16(np.asarray(qkv_w, np.float32), 4)
    projw16 = _w16(np.asarray(proj_w, np.float32), 4)
    w116 = _w16(w1f, 4)
    w216 = _w16(np.asarray(mlp_w2, np.float32), 16)
    qkvb = np.asarray(qkv_b, np.float32)
    qkb = np.ascontiguousarray(qkvb[:2 * DIM])
    vb = np.ascontiguousarray(qkvb[2 * DIM:])
    projb = np.asarray(proj_b, np.float32)
    b2 = np.asarray(mlp_b2, np.float32)

    in_maps = []
    nb = x.shape[0]
    for b in range(nb):
        in_maps.append({
            "x": np.ascontiguousarray(x[b]),
            "arow": np.ascontiguousarray(arow[b]),
            "crow": np.ascontiguousarray(crow[b]),
            "qkvw": qkvw16, "qkb": qkb, "vb": vb,
            "projw": projw16, "projb": projb,
            "w1": w116, "b1": b1f, "w2": w216, "b2": b2,
        })
    return in_maps


def kernel(**inputs):
    in_maps = _prep_in_maps(inputs)
    n_tok = in_maps[0]["x"].shape[0]
    nc = _get_nc(n_tok)
    nb = len(in_maps)
    res = bass_utils.run_bass_kernel_spmd(nc, in_maps, core_ids=list(range(nb)))
    out = np.stack([res.results[b]["out"] for b in range(nb)], axis=0)
    return out
